# revision 17
# baseline (speedup 1.0000x reference)
"""CameraOnlyBEV Trainium2 Bass kernel (8 NeuronCores, data-parallel over B x H).

Sharding: core c handles batch b = c//2, row-half h = c%2 (rows h*32..h*32+32 of
the 64-row feature map) for conv/BN/softmax, and points [h*75000,(h+1)*75000) of
batch b for lift/splat.

- conv1 (3x3, 256->128) = 18 accumulating fp32r matmuls per 512-position chunk
  (9 taps x 2 K-halves) over a zero-padded width-178 layout.
- BN training-mode batch stats are global over (B,H,W): per-core partial
  (sum, sum_sq) + one 1KB AllReduce across the 8 cores.
- depth = mean(softmax).mean() == 1/64 exactly (softmax sums to 1), so the
  point pipeline decouples from the conv pipeline and runs during the
  collective.
- BEV splat: exact occupancy semantics (clip(scatter_add(1),0,1)). All points
  of a shard land in a tiny cell bbox (1/64 depth scale maps everything near
  grid center); the kernel computes the exact bbox min and probes a 3x3 cell
  window with compare+count. A flag output lets the host fall back to an exact
  numpy splat if the bbox ever exceeds the window (never for this input
  distribution), so the kernel is correct for any input.
"""
import numpy as np
from contextlib import ExitStack

import concourse.bass as bass
import concourse.tile as tile
from concourse import bacc, mybir
from concourse.bass_utils import run_bass_kernel_spmd

F32 = mybir.dt.float32
F32R = mybir.dt.float32r
I32 = mybir.dt.int32
ALU = mybir.AluOpType
ACTF = mybir.ActivationFunctionType
AXL = mybir.AxisListType

N_CORES = 8
B, C_IN, FH, FW = 4, 256, 64, 176
N_PTS = 150000
D = 64
HW_ROWS = 32
WPAD = 178
QTOT = HW_ROWS * WPAD          # 5696
NCHUNK = (QTOT + 511) // 512   # 12
TOTAL_POS = B * FH * FW        # 45056
PPC = N_PTS // 2               # 75000 points per core
PPP = 587                      # points per partition (587*128 = 75136)
PPC_PAD = PPP * 128
EPS = 1e-5
DEPTH = float(np.float32(1.0 / 64.0))
PROBE = 3
BEV_COLS = 313                 # 128*313 = 40064 >= 40000


def _build():
    nc = bacc.Bacc("TRN2", target_bir_lowering=False, debug=False,
                   num_devices=N_CORES)

    xpad = nc.dram_tensor("xpad", [C_IN, 34 * WPAD], F32R, kind="ExternalInput").ap()
    uv = nc.dram_tensor("uv", [PPC_PAD, 2], F32, kind="ExternalInput").ap()
    kinv = nc.dram_tensor("kinv", [1, 9], F32, kind="ExternalInput").ap()
    w1t = nc.dram_tensor("w1t", [18, 128, 128], F32R, kind="ExternalInput").ap()
    w2t = nc.dram_tensor("w2t", [128, 64], F32R, kind="ExternalInput").ap()
    bnp = nc.dram_tensor("bnp", [128, 3], F32, kind="ExternalInput").ap()
    b2b = nc.dram_tensor("b2b", [64, 2], F32, kind="ExternalInput").ap()

    dd_o = nc.dram_tensor("dd", [64, HW_ROWS * FW], F32, kind="ExternalOutput").ap()
    ed_o = nc.dram_tensor("ed", [1, HW_ROWS * FW], F32, kind="ExternalOutput").ap()
    pts_o = nc.dram_tensor("pts", [PPC_PAD, 3], F32, kind="ExternalOutput").ap()
    bev_o = nc.dram_tensor("bev", [128, BEV_COLS], F32, kind="ExternalOutput").ap()
    flg_o = nc.dram_tensor("flg", [1, 4], F32, kind="ExternalOutput").ap()

    cc_in = nc.dram_tensor("cc_in", [128, 2], F32)
    cc_out = nc.dram_tensor("cc_out", [128, 2], F32, addr_space="Shared")

    with tile.TileContext(nc) as tc, ExitStack() as ctx:
        cpool = ctx.enter_context(tc.tile_pool(name="const", bufs=1))
        psum = ctx.enter_context(tc.tile_pool(name="psum", bufs=2, space="PSUM"))
        psmall = ctx.enter_context(tc.tile_pool(name="psum_s", bufs=2, space="PSUM"))
        wpool = ctx.enter_context(tc.tile_pool(name="w", bufs=1))
        hpool = ctx.enter_context(tc.tile_pool(name="h", bufs=1))
        ctx2 = ctx.enter_context(ExitStack())

        # ---------------- constants ----------------
        ones_r = cpool.tile([1, 128], F32)
        nc.vector.memset(ones_r[:], 1.0)
        ones_c = cpool.tile([128, 1], F32)
        nc.vector.memset(ones_c[:], 1.0)
        ones64f = cpool.tile([64, 1], F32)
        nc.vector.memset(ones64f[:], 1.0)
        ones64r = cpool.tile([64, 1], F32R)
        nc.vector.tensor_copy(ones64r[:], ones64f[:])
        onesrow_f = cpool.tile([1, 64], F32)
        nc.vector.memset(onesrow_f[:], 1.0)
        onesrow_r = cpool.tile([1, 64], F32R)
        nc.vector.tensor_copy(onesrow_r[:], onesrow_f[:])
        ident = cpool.tile([128, 128], F32)
        nc.vector.memset(ident[:], 1.0)
        nc.gpsimd.affine_select(ident[:], ident[:], pattern=[[-1, 128]],
                                compare_op=ALU.is_equal, fill=0.0,
                                base=0, channel_multiplier=1)

        # ---------------- point pipeline (runs under conv1 + collective) ---
        ppool = ctx2.enter_context(tc.tile_pool(name="pts", bufs=1))
        kin_sb = cpool.tile([1, 9], F32)
        nc.sync.dma_start(kin_sb[:], kinv)
        kb_ps = psmall.tile([128, 9], F32, tag="aux")
        nc.tensor.matmul(kb_ps[:], ones_r[:], kin_sb[:], start=True, stop=True)
        kb = cpool.tile([128, 9], F32)
        nc.scalar.copy(kb[:], kb_ps[:])

        uv_t = ppool.tile([128, PPP, 2], F32)
        nc.sync.dma_start(uv_t[:], uv.rearrange("(p c) k -> p c k", p=128))
        u = uv_t[:, :, 0]
        v = uv_t[:, :, 1]

        xyz = ppool.tile([128, PPP, 3], F32)
        t0 = ppool.tile([128, PPP], F32)
        gxf = ppool.tile([128, PPP], F32)
        gyf = ppool.tile([128, PPP], F32)
        flat = ppool.tile([128, PPP], F32)

        # z (j=2): plain two-step (loose tolerance)
        nc.vector.tensor_scalar(t0[:], u, kb[:, 6:7], None, ALU.mult)
        nc.vector.scalar_tensor_tensor(t0[:], v, kb[:, 7:8], t0[:],
                                       ALU.mult, ALU.add)
        nc.vector.tensor_scalar(xyz[:, :, 2], t0[:], kb[:, 8:9], DEPTH,
                                ALU.add, ALU.mult)

        # x, y (j=0,1): r2 = fma(v, K[j,1], u*K[j,0]) emulated via Dekker so
        # the cell assignment matches the reference's XLA fma chain bitwise.
        SPLITC = 4097.0
        bh = ppool.tile([128, PPP], F32)
        bl = ppool.tile([128, PPP], F32)
        nc.vector.tensor_scalar(bh[:], v, SPLITC, None, ALU.mult)     # tb
        nc.vector.tensor_tensor(bl[:], bh[:], v, ALU.subtract)        # tb - v
        nc.vector.tensor_tensor(bh[:], bh[:], bl[:], ALU.subtract)    # bh
        nc.vector.tensor_tensor(bl[:], v, bh[:], ALU.subtract)        # bl
        ksp = ppool.tile([128, 6], F32)  # per-j: [ah, al, ta] x2
        for j in range(2):
            a = kb[:, 3 * j + 1:3 * j + 2]
            ta = ksp[:, 3 * j + 2:3 * j + 3]
            ah = ksp[:, 3 * j:3 * j + 1]
            al = ksp[:, 3 * j + 1:3 * j + 2]
            nc.vector.tensor_scalar(ta, a, SPLITC, None, ALU.mult)
            nc.vector.tensor_tensor(ah, ta, a, ALU.subtract)
            nc.vector.tensor_tensor(ah, ta, ah, ALU.subtract)
            nc.vector.tensor_tensor(al, a, ah, ALU.subtract)
        e1 = ppool.tile([128, PPP], F32)
        e2 = ppool.tile([128, PPP], F32)
        pp = ppool.tile([128, PPP], F32)
        for j in range(2):
            a = kb[:, 3 * j + 1:3 * j + 2]
            ah = ksp[:, 3 * j:3 * j + 1]
            al = ksp[:, 3 * j + 1:3 * j + 2]
            nc.vector.tensor_scalar(pp[:], v, a, None, ALU.mult)          # p
            nc.vector.tensor_scalar(e1[:], bh[:], ah, None, ALU.mult)     # bh*ah
            nc.vector.tensor_tensor(e1[:], e1[:], pp[:], ALU.subtract)
            nc.vector.tensor_scalar(e2[:], bh[:], al, None, ALU.mult)
            nc.vector.tensor_tensor(e1[:], e1[:], e2[:], ALU.add)
            nc.vector.tensor_scalar(e2[:], bl[:], ah, None, ALU.mult)
            nc.vector.tensor_tensor(e1[:], e1[:], e2[:], ALU.add)
            nc.vector.tensor_scalar(e2[:], bl[:], al, None, ALU.mult)
            nc.vector.tensor_tensor(e1[:], e1[:], e2[:], ALU.add)         # err
            nc.vector.tensor_scalar(t0[:], u, kb[:, 3 * j:3 * j + 1],
                                    None, ALU.mult)                       # r1
            # TwoSum(r1, p)
            s_ = gxf if j == 0 else gyf  # reuse as scratch for s
            nc.vector.tensor_tensor(s_[:], t0[:], pp[:], ALU.add)         # s
            nc.vector.tensor_tensor(e2[:], s_[:], t0[:], ALU.subtract)    # bb
            nc.vector.tensor_tensor(flat[:], s_[:], e2[:], ALU.subtract)  # s-bb
            nc.vector.tensor_tensor(flat[:], t0[:], flat[:], ALU.subtract)  # t-(s-bb)
            nc.vector.tensor_tensor(e2[:], pp[:], e2[:], ALU.subtract)    # p-bb
            nc.vector.tensor_tensor(e2[:], flat[:], e2[:], ALU.add)       # ee
            nc.vector.tensor_tensor(e1[:], e1[:], e2[:], ALU.add)         # err+ee
            nc.vector.tensor_tensor(s_[:], s_[:], e1[:], ALU.add)         # r2
            nc.vector.tensor_scalar(xyz[:, :, j], s_[:],
                                    kb[:, 3 * j + 2:3 * j + 3], DEPTH,
                                    ALU.add, ALU.mult)
        nc.sync.dma_start(pts_o.rearrange("(p c) k -> p (c k)", p=128),
                          xyz[:].rearrange("p c k -> p (c k)"))

        vi = ppool.tile([128, PPP], I32)
        cf = ppool.tile([128, PPP], F32)
        for src_j, gout in ((0, gxf), (1, gyf)):
            nc.vector.tensor_scalar(t0[:], xyz[:, :, src_j], 50.0, 2.0,
                                    ALU.add, ALU.mult)
            nc.vector.tensor_scalar(t0[:], t0[:], 0.0, 199.0, ALU.max, ALU.min)
            nc.vector.tensor_copy(vi[:], t0[:])
            nc.vector.tensor_copy(cf[:], vi[:])
            nc.vector.tensor_tensor(gout[:], cf[:], t0[:], ALU.is_gt)
            nc.vector.tensor_tensor(gout[:], cf[:], gout[:], ALU.subtract)
        nc.vector.scalar_tensor_tensor(flat[:], gyf[:], 200.0, gxf[:],
                                       ALU.mult, ALU.add)

        st = ppool.tile([128, 4], F32)
        stn = ppool.tile([128, 2], F32)
        nc.vector.tensor_reduce(stn[:, 0:1], gxf[:], axis=AXL.X, op=ALU.min)
        nc.vector.tensor_reduce(st[:, 1:2], gxf[:], axis=AXL.X, op=ALU.max)
        nc.vector.tensor_reduce(stn[:, 1:2], gyf[:], axis=AXL.X, op=ALU.min)
        nc.vector.tensor_reduce(st[:, 3:4], gyf[:], axis=AXL.X, op=ALU.max)
        nc.vector.tensor_scalar(st[:, 0:1], stn[:, 0:1], -1.0, None, ALU.mult)
        nc.vector.tensor_scalar(st[:, 2:3], stn[:, 1:2], -1.0, None, ALU.mult)
        stt_ps = psmall.tile([4, 128], F32, tag="aux")
        nc.tensor.transpose(stt_ps[:], st[:], ident[:])
        gst = ppool.tile([4, 1], F32)
        nc.vector.tensor_reduce(gst[:], stt_ps[:], axis=AXL.X, op=ALU.max)
        g_ps = psmall.tile([1, 4], F32, tag="aux")
        nc.tensor.matmul(g_ps[:], gst[:, 0:1], ident[0:4, 0:4],
                         start=True, stop=True)
        g_row = ppool.tile([1, 4], F32)
        nc.scalar.copy(g_row[:], g_ps[:])
        nc.sync.dma_start(flg_o, g_row[:])

        base1 = ppool.tile([1, 1], F32)
        nc.vector.scalar_tensor_tensor(base1[:], g_row[:, 2:3], 200.0,
                                       g_row[:, 0:1], ALU.mult, ALU.add)
        nc.vector.tensor_scalar(base1[:], base1[:], -1.0, None, ALU.mult)
        bb_ps = psmall.tile([128, 1], F32, tag="aux")
        nc.tensor.matmul(bb_ps[:], ones_r[:], base1[:], start=True, stop=True)
        baseb = ppool.tile([128, 1], F32)
        nc.scalar.copy(baseb[:], bb_ps[:])

        rel = ppool.tile([128, PPP], F32)
        nc.vector.tensor_scalar(rel[:], flat[:], baseb[:, 0:1], None, ALU.subtract)
        hits = ppool.tile([128, PROBE * PROBE], F32)
        scratch = ppool.tile([128, PPP], F32)
        for i in range(PROBE * PROBE):
            off = float((i // PROBE) * 200 + (i % PROBE))
            nc.vector.tensor_scalar(scratch[:], rel[:], off, 0.0,
                                    ALU.is_equal, ALU.add,
                                    accum_out=hits[:, i:i + 1])
        cnt_ps = psmall.tile([1, PROBE * PROBE], F32, tag="aux")
        nc.tensor.matmul(cnt_ps[:], ones_c[:], hits[:], start=True, stop=True)
        occ1 = ppool.tile([1, PROBE * PROBE], F32)
        nc.vector.tensor_scalar(occ1[:], cnt_ps[:], 0.0, None, ALU.is_gt)
        ob_ps = psmall.tile([128, PROBE * PROBE], F32, tag="aux")
        nc.tensor.matmul(ob_ps[:], ones_r[:], occ1[:], start=True, stop=True)
        occb = ppool.tile([128, PROBE * PROBE], F32)
        nc.scalar.copy(occb[:], ob_ps[:])

        cell_i = ppool.tile([128, BEV_COLS], I32)
        nc.gpsimd.iota(cell_i[:], pattern=[[1, BEV_COLS]], base=0,
                       channel_multiplier=BEV_COLS)
        cell_f = ppool.tile([128, BEV_COLS], F32)
        nc.vector.tensor_copy(cell_f[:], cell_i[:])
        relc = ppool.tile([128, BEV_COLS], F32)
        nc.vector.tensor_scalar(relc[:], cell_f[:], baseb[:, 0:1], None,
                                ALU.subtract)
        bev = ppool.tile([128, BEV_COLS], F32)
        nc.vector.memset(bev[:], 0.0)
        mk = ppool.tile([128, BEV_COLS], F32)
        for i in range(PROBE * PROBE):
            off = float((i // PROBE) * 200 + (i % PROBE))
            nc.vector.tensor_scalar(mk[:], relc[:], off, None, ALU.is_equal)
            nc.vector.scalar_tensor_tensor(bev[:], mk[:], occb[:, i:i + 1],
                                           bev[:], ALU.mult, ALU.add)
        nc.sync.dma_start(bev_o, bev[:])

        # ---------------- conv1 ----------------
        xpool = ctx2.enter_context(tc.tile_pool(name="x", bufs=1))
        wts = wpool.tile([128, 18, 128], F32R)
        nc.sync.dma_start(wts[:], w1t.rearrange("t a b -> a t b"))

        xk = []
        for kh in range(2):
            xt = xpool.tile([128, 34 * WPAD + 2], F32R, tag=f"xk{kh}")
            zr2 = xpool.tile([128, 2], F32, tag="zr2")
            nc.vector.memset(zr2[:], 0.0)
            nc.vector.tensor_copy(xt[:, 34 * WPAD:], zr2[:])
            nc.sync.dma_start(xt[:, 0:34 * WPAD], xpad[kh * 128:(kh + 1) * 128, :])
            xk.append(xt)

        h_raw = hpool.tile([128, QTOT], F32)
        zero128 = cpool.tile([128, 1], F32)
        nc.vector.memset(zero128[:], 0.0)
        sumh_c = hpool.tile([128, NCHUNK], F32)
        sumq_c = hpool.tile([128, NCHUNK], F32)
        sq_scr = hpool.tile([128, 512], F32)

        for c in range(NCHUNK):
            q0 = c * 512
            n = min(512, QTOT - q0)
            ps = psum.tile([128, 512], F32, tag="big")
            k = 0
            for dy in range(3):
                for dx in range(3):
                    off = dy * WPAD + dx
                    for kh in range(2):
                        nc.tensor.matmul(
                            ps[:, 0:n],
                            wts[:, (dy * 3 + dx) * 2 + kh, :],
                            xk[kh][:, q0 + off:q0 + off + n],
                            start=(k == 0), stop=(k == 17),
                        )
                        k += 1
            nc.scalar.activation(h_raw[:, q0:q0 + n], ps[:, 0:n], ACTF.Copy,
                                 accum_out=sumh_c[:, c:c + 1])
            nc.scalar.activation(sq_scr[:, 0:n], ps[:, 0:n], ACTF.Square,
                                 bias=zero128[:], accum_out=sumq_c[:, c:c + 1])

        # stats correction for the 2 garbage cols per padded row
        garb = h_raw[:].rearrange("p (h w) -> p h w", h=HW_ROWS)[:, :, FW:WPAD]
        gsum = hpool.tile([128, 1], F32)
        nc.vector.tensor_reduce(gsum[:], garb, axis=AXL.XY, op=ALU.add)
        gsq_scr = hpool.tile([128, 64], F32)
        gsq = hpool.tile([128, 1], F32)
        nc.scalar.activation(gsq_scr[:], garb, ACTF.Square, bias=zero128[:],
                             accum_out=gsq[:])

        stats = hpool.tile([128, 2], F32)
        nc.vector.tensor_reduce(stats[:, 0:1], sumh_c[:], axis=AXL.X, op=ALU.add)
        nc.vector.tensor_reduce(stats[:, 1:2], sumq_c[:], axis=AXL.X, op=ALU.add)
        nc.vector.tensor_tensor(stats[:, 0:1], stats[:, 0:1], gsum[:], ALU.subtract)
        nc.vector.tensor_tensor(stats[:, 1:2], stats[:, 1:2], gsq[:], ALU.subtract)

        nc.sync.dma_start(cc_in.ap(), stats[:])
        nc.gpsimd.collective_compute(
            "AllReduce", ALU.add,
            replica_groups=[list(range(N_CORES))],
            ins=[cc_in.ap()], outs=[cc_out.ap()],
        )

        # ---------------- BN + conv2 + softmax ----------------
        ctx2.close()
        spool = ctx.enter_context(tc.tile_pool(name="smax", bufs=1))
        gstats = hpool.tile([128, 2], F32)
        nc.sync.dma_start(gstats[:], cc_out.ap())

        bn_sb = cpool.tile([128, 3], F32)
        nc.sync.dma_start(bn_sb[:], bnp)
        mu = hpool.tile([128, 1], F32)
        nc.vector.tensor_scalar(mu[:], gstats[:, 0:1], 1.0 / TOTAL_POS, None, ALU.mult)
        var = hpool.tile([128, 1], F32)
        nc.vector.tensor_scalar(var[:], gstats[:, 1:2], 1.0 / TOTAL_POS, None, ALU.mult)
        mu2 = hpool.tile([128, 1], F32)
        nc.vector.tensor_tensor(mu2[:], mu[:], mu[:], ALU.mult)
        nc.vector.tensor_tensor(var[:], var[:], mu2[:], ALU.subtract)
        sd = hpool.tile([128, 1], F32)
        eps_t = hpool.tile([128, 1], F32)
        nc.vector.memset(eps_t[:], float(EPS))
        nc.scalar.activation(sd[:], var[:], ACTF.Sqrt, bias=eps_t[:])
        rinv = hpool.tile([128, 1], F32)
        nc.vector.reciprocal(rinv[:], sd[:])
        scale = hpool.tile([128, 1], F32)
        nc.vector.tensor_tensor(scale[:], rinv[:], bn_sb[:, 1:2], ALU.mult)
        bias_f = hpool.tile([128, 1], F32)
        nc.vector.tensor_tensor(bias_f[:], mu[:], bn_sb[:, 0:1], ALU.add)
        nc.vector.tensor_tensor(bias_f[:], bias_f[:], scale[:], ALU.mult)
        nc.vector.tensor_tensor(bias_f[:], bn_sb[:, 2:3], bias_f[:], ALU.subtract)

        h_relu = hpool.tile([128, QTOT], F32R)
        nc.scalar.activation(h_relu[:], h_raw[:], ACTF.Relu,
                             bias=bias_f[:], scale=scale[:])

        w2_sb = cpool.tile([128, 64], F32R)
        nc.sync.dma_start(w2_sb[:], w2t)
        b2_sb = cpool.tile([64, 2], F32)
        nc.sync.dma_start(b2_sb[:], b2b)
        bins_r = cpool.tile([64, 1], F32R)
        nc.vector.tensor_copy(bins_r[:], b2_sb[:, 1:2])

        exp_t = spool.tile([64, QTOT], F32R)
        den = spool.tile([1, QTOT], F32R)
        num = spool.tile([1, QTOT], F32)
        for c in range(NCHUNK):
            q0 = c * 512
            n = min(512, QTOT - q0)
            ps2 = psum.tile([64, 512], F32, tag="big")
            nc.tensor.matmul(ps2[:, 0:n], w2_sb[:], h_relu[:, q0:q0 + n],
                             start=True, stop=True)
            nc.scalar.activation(exp_t[:, q0:q0 + n], ps2[:, 0:n], ACTF.Exp,
                                 bias=b2_sb[:, 0:1], scale=1.0)
            psd = psmall.tile([1, 512], F32, tag="dn")
            nc.tensor.matmul(psd[:, 0:n], ones64r[:], exp_t[:, q0:q0 + n],
                             start=True, stop=True)
            nc.scalar.copy(den[:, q0:q0 + n], psd[:, 0:n])
            psn = psmall.tile([1, 512], F32, tag="dn")
            nc.tensor.matmul(psn[:, 0:n], bins_r[:], exp_t[:, q0:q0 + n],
                             start=True, stop=True)
            nc.scalar.copy(num[:, q0:q0 + n], psn[:, 0:n])

        rden = spool.tile([1, QTOT], F32)
        nc.vector.reciprocal(rden[:], den[:].bitcast(F32))
        rden_r = spool.tile([1, QTOT], F32R)
        nc.scalar.copy(rden_r[:], rden[:])
        ed1 = num
        nc.vector.tensor_tensor(ed1[:], num[:], rden[:], ALU.mult)
        nc.sync.dma_start(
            ed_o.rearrange("one (h w) -> one h w", h=HW_ROWS),
            ed1[:].rearrange("one (h w) -> one h w", h=HW_ROWS)[:, :, 0:FW])

        probs = spool.tile([64, QTOT], F32)
        for c in range(NCHUNK):
            q0 = c * 512
            n = min(512, QTOT - q0)
            psr = psum.tile([64, 512], F32, tag="big")
            nc.tensor.matmul(psr[:, 0:n], onesrow_r[:], rden_r[:, q0:q0 + n],
                             start=True, stop=True)
            nc.vector.tensor_tensor(probs[:, q0:q0 + n],
                                    exp_t[:, q0:q0 + n].bitcast(F32),
                                    psr[:, 0:n], ALU.mult)
        nc.sync.dma_start(
            dd_o.rearrange("d (h w) -> d h w", h=HW_ROWS),
            probs[:].rearrange("d (h w) -> d h w", h=HW_ROWS)[:, :, 0:FW])

    nc.compile()
    return nc


_NC_CACHE = None


def kernel(camera_features, pixels_uv, K_inv, W1, b1, gamma, beta, W2, b2,
           depth_bins):
    global _NC_CACHE
    if _NC_CACHE is None:
        _NC_CACHE = _build()
    nc = _NC_CACHE

    camera_features = np.asarray(camera_features, dtype=np.float32)
    pixels_uv = np.ascontiguousarray(np.asarray(pixels_uv, dtype=np.float32))
    K_inv = np.asarray(K_inv, dtype=np.float32)
    W1 = np.asarray(W1, dtype=np.float32)
    b1 = np.asarray(b1, dtype=np.float32)
    gamma = np.asarray(gamma, dtype=np.float32)
    beta = np.asarray(beta, dtype=np.float32)
    W2 = np.asarray(W2, dtype=np.float32)
    b2 = np.asarray(b2, dtype=np.float32)
    depth_bins = np.asarray(depth_bins, dtype=np.float32)

    # host-side layout prep (pure data movement)
    w1t = np.empty((18, 128, 128), np.float32)
    for ky in range(3):
        for kx in range(3):
            for kh in range(2):
                # [ci, co] for tap (ky,kx), K-half kh
                w1t[(ky * 3 + kx) * 2 + kh] = \
                    W1[:, kh * 128:(kh + 1) * 128, ky, kx].T
    xp = np.zeros((B, C_IN, FH + 2, WPAD), np.float32)
    xp[:, :, 1:FH + 1, 1:FW + 1] = camera_features
    w2t = np.ascontiguousarray(W2[:, :, 0, 0].T)
    bnp = np.ascontiguousarray(np.stack([b1, gamma, beta], axis=1))
    b2bins = np.ascontiguousarray(np.stack([b2, depth_bins], axis=1))

    in_maps = []
    for c in range(N_CORES):
        b = c // 2
        half = c % 2
        r0 = half * HW_ROWS
        uv_sl = pixels_uv[b, half * PPC:(half + 1) * PPC]
        uv_sh = np.concatenate([uv_sl, uv_sl[:PPC_PAD - PPC]], axis=0)
        in_maps.append({
            "xpad": np.ascontiguousarray(
                xp[b, :, r0:r0 + 34, :]).reshape(C_IN, 34 * WPAD),
            "uv": np.ascontiguousarray(uv_sh),
            "kinv": K_inv[b].reshape(1, 9).copy(),
            "w1t": w1t,
            "w2t": w2t,
            "bnp": bnp,
            "b2b": b2bins,
        })

    trace = bool(getattr(kernel, "_trace", False))
    res = run_bass_kernel_spmd(nc, in_maps, core_ids=list(range(N_CORES)),
                               trace=trace)
    kernel._last_exec_ns = res.exec_time_ns
    kernel._last_results = res

    dd = np.empty((B, D, FH, FW), np.float32)
    ed = np.empty((B, FH, FW), np.float32)
    pts = np.empty((B, N_PTS, 3), np.float32)
    bev = np.empty((B, 200, 200), np.float32)
    fallback = False
    for c in range(N_CORES):
        b = c // 2
        half = c % 2
        r0 = half * HW_ROWS
        r = res.results[c]
        dd[b, :, r0:r0 + HW_ROWS, :] = r["dd"].reshape(D, HW_ROWS, FW)
        ed[b, r0:r0 + HW_ROWS, :] = r["ed"].reshape(HW_ROWS, FW)
        pts[b, half * PPC:(half + 1) * PPC] = r["pts"][:PPC]
        g = r["bev"].ravel()[:40000].reshape(200, 200)
        flg = r["flg"].ravel()  # [-minx, maxx, -miny, maxy]
        if (flg[1] + flg[0] > PROBE - 1) or (flg[3] + flg[2] > PROBE - 1):
            fallback = True
        if half == 0:
            bev[b] = g
        else:
            np.maximum(bev[b], g, out=bev[b])

    if fallback:
        # exact host fallback (never taken for the target input distribution)
        for b in range(B):
            gx = np.clip(((pts[b, :, 0] + 50.0) / 0.5).astype(np.int32), 0, 199)
            gy = np.clip(((pts[b, :, 1] + 50.0) / 0.5).astype(np.int32), 0, 199)
            grid = np.zeros(40000, np.float32)
            np.add.at(grid, gy * 200 + gx, 1.0)
            bev[b] = np.clip(grid, 0.0, 1.0).reshape(200, 200)

    return bev, dd, ed, pts


# revision 18
# speedup vs baseline: 1.0101x; 1.0101x over previous
"""CameraOnlyBEV Trainium2 Bass kernel (8 NeuronCores, data-parallel over B x H).

Sharding: core c handles batch b = c//2, row-half h = c%2 (rows h*32..h*32+32 of
the 64-row feature map) for conv/BN/softmax, and points [h*75000,(h+1)*75000) of
batch b for lift/splat.

- conv1 (3x3, 256->128) = 18 accumulating fp32r matmuls per 512-position chunk
  (9 taps x 2 K-halves) over a zero-padded width-178 layout.
- BN training-mode batch stats are global over (B,H,W): per-core partial
  (sum, sum_sq) + one 1KB AllReduce across the 8 cores.
- depth = mean(softmax).mean() == 1/64 exactly (softmax sums to 1), so the
  point pipeline decouples from the conv pipeline and runs during the
  collective.
- BEV splat: exact occupancy semantics (clip(scatter_add(1),0,1)). All points
  of a shard land in a tiny cell bbox (1/64 depth scale maps everything near
  grid center); the kernel computes the exact bbox min and probes a 3x3 cell
  window with compare+count. A flag output lets the host fall back to an exact
  numpy splat if the bbox ever exceeds the window (never for this input
  distribution), so the kernel is correct for any input.
"""
import numpy as np
from contextlib import ExitStack

import concourse.bass as bass
import concourse.tile as tile
from concourse import bacc, mybir
from concourse.bass_utils import run_bass_kernel_spmd

F32 = mybir.dt.float32
F32R = mybir.dt.float32r
I32 = mybir.dt.int32
ALU = mybir.AluOpType
ACTF = mybir.ActivationFunctionType
AXL = mybir.AxisListType

N_CORES = 8
B, C_IN, FH, FW = 4, 256, 64, 176
N_PTS = 150000
D = 64
HW_ROWS = 32
WPAD = 178
QTOT = HW_ROWS * WPAD          # 5696
NCHUNK = (QTOT + 511) // 512   # 12
TOTAL_POS = B * FH * FW        # 45056
PPC = N_PTS // 2               # 75000 points per core
PPP = 587                      # points per partition (587*128 = 75136)
PPC_PAD = PPP * 128
EPS = 1e-5
DEPTH = float(np.float32(1.0 / 64.0))
PROBE = 3
BEV_COLS = 313                 # 128*313 = 40064 >= 40000


def _build():
    nc = bacc.Bacc("TRN2", target_bir_lowering=False, debug=False,
                   num_devices=N_CORES)

    xpad = nc.dram_tensor("xpad", [C_IN, 34 * WPAD], F32R, kind="ExternalInput").ap()
    uv = nc.dram_tensor("uv", [PPC_PAD, 2], F32, kind="ExternalInput").ap()
    kinv = nc.dram_tensor("kinv", [1, 9], F32, kind="ExternalInput").ap()
    w1t = nc.dram_tensor("w1t", [18, 128, 128], F32R, kind="ExternalInput").ap()
    w2t = nc.dram_tensor("w2t", [128, 64], F32R, kind="ExternalInput").ap()
    bnp = nc.dram_tensor("bnp", [128, 3], F32, kind="ExternalInput").ap()
    b2b = nc.dram_tensor("b2b", [64, 2], F32, kind="ExternalInput").ap()

    dd_o = nc.dram_tensor("dd", [64, HW_ROWS * FW], F32, kind="ExternalOutput").ap()
    ed_o = nc.dram_tensor("ed", [1, HW_ROWS * FW], F32, kind="ExternalOutput").ap()
    pts_o = nc.dram_tensor("pts", [PPC_PAD, 3], F32, kind="ExternalOutput").ap()
    bev_o = nc.dram_tensor("bev", [128, BEV_COLS], F32, kind="ExternalOutput").ap()
    flg_o = nc.dram_tensor("flg", [1, 4], F32, kind="ExternalOutput").ap()

    cc_in = nc.dram_tensor("cc_in", [128, 2], F32)
    cc_out = nc.dram_tensor("cc_out", [128, 2], F32, addr_space="Shared")

    with tile.TileContext(nc) as tc, ExitStack() as ctx:
        cpool = ctx.enter_context(tc.tile_pool(name="const", bufs=1))
        psum = ctx.enter_context(tc.tile_pool(name="psum", bufs=2, space="PSUM"))
        psmall = ctx.enter_context(tc.tile_pool(name="psum_s", bufs=2, space="PSUM"))
        wpool = ctx.enter_context(tc.tile_pool(name="w", bufs=1))
        hpool = ctx.enter_context(tc.tile_pool(name="h", bufs=1))
        ctx2 = ctx.enter_context(ExitStack())

        # ---------------- constants ----------------
        ones_r = cpool.tile([1, 128], F32)
        nc.vector.memset(ones_r[:], 1.0)
        ones_c = cpool.tile([128, 1], F32)
        nc.vector.memset(ones_c[:], 1.0)
        ones64f = cpool.tile([64, 1], F32)
        nc.vector.memset(ones64f[:], 1.0)
        ones64r = cpool.tile([64, 1], F32R)
        nc.vector.tensor_copy(ones64r[:], ones64f[:])
        onesrow_f = cpool.tile([1, 64], F32)
        nc.vector.memset(onesrow_f[:], 1.0)
        onesrow_r = cpool.tile([1, 64], F32R)
        nc.vector.tensor_copy(onesrow_r[:], onesrow_f[:])
        ident = cpool.tile([128, 128], F32)
        nc.vector.memset(ident[:], 1.0)
        nc.gpsimd.affine_select(ident[:], ident[:], pattern=[[-1, 128]],
                                compare_op=ALU.is_equal, fill=0.0,
                                base=0, channel_multiplier=1)

        # ---------------- conv1 ----------------
        ppool = ctx2.enter_context(tc.tile_pool(name="pts", bufs=1))
        xpool = ctx2.enter_context(tc.tile_pool(name="x", bufs=1))
        wts = wpool.tile([128, 18, 128], F32R)
        nc.sync.dma_start(wts[:], w1t.rearrange("t a b -> a t b"))

        xk = []
        for kh in range(2):
            xt = xpool.tile([128, 34 * WPAD + 2], F32R, tag=f"xk{kh}")
            zr2 = xpool.tile([128, 2], F32, tag="zr2")
            nc.vector.memset(zr2[:], 0.0)
            nc.vector.tensor_copy(xt[:, 34 * WPAD:], zr2[:])
            nc.sync.dma_start(xt[:, 0:34 * WPAD], xpad[kh * 128:(kh + 1) * 128, :])
            xk.append(xt)

        h_raw = hpool.tile([128, QTOT], F32)
        zero128 = cpool.tile([128, 1], F32)
        nc.vector.memset(zero128[:], 0.0)
        sumh_c = hpool.tile([128, NCHUNK], F32)
        sumq_c = hpool.tile([128, NCHUNK], F32)
        sq_scr = hpool.tile([128, 512], F32)

        for c in range(NCHUNK):
            q0 = c * 512
            n = min(512, QTOT - q0)
            ps = psum.tile([128, 512], F32, tag="big")
            k = 0
            for dy in range(3):
                for dx in range(3):
                    off = dy * WPAD + dx
                    for kh in range(2):
                        nc.tensor.matmul(
                            ps[:, 0:n],
                            wts[:, (dy * 3 + dx) * 2 + kh, :],
                            xk[kh][:, q0 + off:q0 + off + n],
                            start=(k == 0), stop=(k == 17),
                        )
                        k += 1
            nc.scalar.activation(h_raw[:, q0:q0 + n], ps[:, 0:n], ACTF.Copy,
                                 accum_out=sumh_c[:, c:c + 1])
            nc.scalar.activation(sq_scr[:, 0:n], ps[:, 0:n], ACTF.Square,
                                 bias=zero128[:], accum_out=sumq_c[:, c:c + 1])

        # stats correction for the 2 garbage cols per padded row
        garb = h_raw[:].rearrange("p (h w) -> p h w", h=HW_ROWS)[:, :, FW:WPAD]
        gsum = hpool.tile([128, 1], F32)
        nc.vector.tensor_reduce(gsum[:], garb, axis=AXL.XY, op=ALU.add)
        gsq_scr = hpool.tile([128, 64], F32)
        gsq = hpool.tile([128, 1], F32)
        nc.scalar.activation(gsq_scr[:], garb, ACTF.Square, bias=zero128[:],
                             accum_out=gsq[:])

        stats = hpool.tile([128, 2], F32)
        nc.vector.tensor_reduce(stats[:, 0:1], sumh_c[:], axis=AXL.X, op=ALU.add)
        nc.vector.tensor_reduce(stats[:, 1:2], sumq_c[:], axis=AXL.X, op=ALU.add)
        nc.vector.tensor_tensor(stats[:, 0:1], stats[:, 0:1], gsum[:], ALU.subtract)
        nc.vector.tensor_tensor(stats[:, 1:2], stats[:, 1:2], gsq[:], ALU.subtract)

        nc.sync.dma_start(cc_in.ap(), stats[:])
        nc.gpsimd.collective_compute(
            "AllReduce", ALU.add,
            replica_groups=[list(range(N_CORES))],
            ins=[cc_in.ap()], outs=[cc_out.ap()],
        )

        # ---------------- point pipeline (overlaps the collective) ---------
        kin_sb = cpool.tile([1, 9], F32)
        nc.sync.dma_start(kin_sb[:], kinv)
        kb_ps = psmall.tile([128, 9], F32, tag="aux")
        nc.tensor.matmul(kb_ps[:], ones_r[:], kin_sb[:], start=True, stop=True)
        kb = cpool.tile([128, 9], F32)
        nc.scalar.copy(kb[:], kb_ps[:])

        uv_t = ppool.tile([128, PPP, 2], F32)
        nc.sync.dma_start(uv_t[:], uv.rearrange("(p c) k -> p c k", p=128))
        u = uv_t[:, :, 0]
        v = uv_t[:, :, 1]

        xyz = ppool.tile([128, PPP, 3], F32)
        t0 = ppool.tile([128, PPP], F32)
        gxf = ppool.tile([128, PPP], F32)
        gyf = ppool.tile([128, PPP], F32)
        flat = ppool.tile([128, PPP], F32)

        # z (j=2): plain two-step (loose tolerance)
        nc.vector.tensor_scalar(t0[:], u, kb[:, 6:7], None, ALU.mult)
        nc.vector.scalar_tensor_tensor(t0[:], v, kb[:, 7:8], t0[:],
                                       ALU.mult, ALU.add)
        nc.vector.tensor_scalar(xyz[:, :, 2], t0[:], kb[:, 8:9], DEPTH,
                                ALU.add, ALU.mult)

        # x, y (j=0,1): r2 = fma(v, K[j,1], u*K[j,0]) emulated via Dekker so
        # the cell assignment matches the reference's XLA fma chain bitwise.
        SPLITC = 4097.0
        bh = ppool.tile([128, PPP], F32)
        bl = ppool.tile([128, PPP], F32)
        nc.vector.tensor_scalar(bh[:], v, SPLITC, None, ALU.mult)     # tb
        nc.vector.tensor_tensor(bl[:], bh[:], v, ALU.subtract)        # tb - v
        nc.vector.tensor_tensor(bh[:], bh[:], bl[:], ALU.subtract)    # bh
        nc.vector.tensor_tensor(bl[:], v, bh[:], ALU.subtract)        # bl
        ksp = ppool.tile([128, 6], F32)  # per-j: [ah, al, ta] x2
        for j in range(2):
            a = kb[:, 3 * j + 1:3 * j + 2]
            ta = ksp[:, 3 * j + 2:3 * j + 3]
            ah = ksp[:, 3 * j:3 * j + 1]
            al = ksp[:, 3 * j + 1:3 * j + 2]
            nc.vector.tensor_scalar(ta, a, SPLITC, None, ALU.mult)
            nc.vector.tensor_tensor(ah, ta, a, ALU.subtract)
            nc.vector.tensor_tensor(ah, ta, ah, ALU.subtract)
            nc.vector.tensor_tensor(al, a, ah, ALU.subtract)
        e1 = ppool.tile([128, PPP], F32)
        e2 = ppool.tile([128, PPP], F32)
        pp = ppool.tile([128, PPP], F32)
        for j in range(2):
            a = kb[:, 3 * j + 1:3 * j + 2]
            ah = ksp[:, 3 * j:3 * j + 1]
            al = ksp[:, 3 * j + 1:3 * j + 2]
            nc.vector.tensor_scalar(pp[:], v, a, None, ALU.mult)          # p
            nc.vector.tensor_scalar(e1[:], bh[:], ah, None, ALU.mult)     # bh*ah
            nc.vector.tensor_tensor(e1[:], e1[:], pp[:], ALU.subtract)
            nc.vector.tensor_scalar(e2[:], bh[:], al, None, ALU.mult)
            nc.vector.tensor_tensor(e1[:], e1[:], e2[:], ALU.add)
            nc.vector.tensor_scalar(e2[:], bl[:], ah, None, ALU.mult)
            nc.vector.tensor_tensor(e1[:], e1[:], e2[:], ALU.add)
            nc.vector.tensor_scalar(e2[:], bl[:], al, None, ALU.mult)
            nc.vector.tensor_tensor(e1[:], e1[:], e2[:], ALU.add)         # err
            nc.vector.tensor_scalar(t0[:], u, kb[:, 3 * j:3 * j + 1],
                                    None, ALU.mult)                       # r1
            # TwoSum(r1, p)
            s_ = gxf if j == 0 else gyf  # reuse as scratch for s
            nc.vector.tensor_tensor(s_[:], t0[:], pp[:], ALU.add)         # s
            nc.vector.tensor_tensor(e2[:], s_[:], t0[:], ALU.subtract)    # bb
            nc.vector.tensor_tensor(flat[:], s_[:], e2[:], ALU.subtract)  # s-bb
            nc.vector.tensor_tensor(flat[:], t0[:], flat[:], ALU.subtract)  # t-(s-bb)
            nc.vector.tensor_tensor(e2[:], pp[:], e2[:], ALU.subtract)    # p-bb
            nc.vector.tensor_tensor(e2[:], flat[:], e2[:], ALU.add)       # ee
            nc.vector.tensor_tensor(e1[:], e1[:], e2[:], ALU.add)         # err+ee
            nc.vector.tensor_tensor(s_[:], s_[:], e1[:], ALU.add)         # r2
            nc.vector.tensor_scalar(xyz[:, :, j], s_[:],
                                    kb[:, 3 * j + 2:3 * j + 3], DEPTH,
                                    ALU.add, ALU.mult)
        nc.sync.dma_start(pts_o.rearrange("(p c) k -> p (c k)", p=128),
                          xyz[:].rearrange("p c k -> p (c k)"))

        vi = ppool.tile([128, PPP], I32)
        cf = ppool.tile([128, PPP], F32)
        for src_j, gout in ((0, gxf), (1, gyf)):
            nc.vector.tensor_scalar(t0[:], xyz[:, :, src_j], 50.0, 2.0,
                                    ALU.add, ALU.mult)
            nc.vector.tensor_scalar(t0[:], t0[:], 0.0, 199.0, ALU.max, ALU.min)
            nc.vector.tensor_copy(vi[:], t0[:])
            nc.vector.tensor_copy(cf[:], vi[:])
            nc.vector.tensor_tensor(gout[:], cf[:], t0[:], ALU.is_gt)
            nc.vector.tensor_tensor(gout[:], cf[:], gout[:], ALU.subtract)
        nc.vector.scalar_tensor_tensor(flat[:], gyf[:], 200.0, gxf[:],
                                       ALU.mult, ALU.add)

        st = ppool.tile([128, 4], F32)
        stn = ppool.tile([128, 2], F32)
        nc.vector.tensor_reduce(stn[:, 0:1], gxf[:], axis=AXL.X, op=ALU.min)
        nc.vector.tensor_reduce(st[:, 1:2], gxf[:], axis=AXL.X, op=ALU.max)
        nc.vector.tensor_reduce(stn[:, 1:2], gyf[:], axis=AXL.X, op=ALU.min)
        nc.vector.tensor_reduce(st[:, 3:4], gyf[:], axis=AXL.X, op=ALU.max)
        nc.vector.tensor_scalar(st[:, 0:1], stn[:, 0:1], -1.0, None, ALU.mult)
        nc.vector.tensor_scalar(st[:, 2:3], stn[:, 1:2], -1.0, None, ALU.mult)
        stt_ps = psmall.tile([4, 128], F32, tag="aux")
        nc.tensor.transpose(stt_ps[:], st[:], ident[:])
        gst = ppool.tile([4, 1], F32)
        nc.vector.tensor_reduce(gst[:], stt_ps[:], axis=AXL.X, op=ALU.max)
        g_ps = psmall.tile([1, 4], F32, tag="aux")
        nc.tensor.matmul(g_ps[:], gst[:, 0:1], ident[0:4, 0:4],
                         start=True, stop=True)
        g_row = ppool.tile([1, 4], F32)
        nc.scalar.copy(g_row[:], g_ps[:])
        nc.sync.dma_start(flg_o, g_row[:])

        base1 = ppool.tile([1, 1], F32)
        nc.vector.scalar_tensor_tensor(base1[:], g_row[:, 2:3], 200.0,
                                       g_row[:, 0:1], ALU.mult, ALU.add)
        nc.vector.tensor_scalar(base1[:], base1[:], -1.0, None, ALU.mult)
        bb_ps = psmall.tile([128, 1], F32, tag="aux")
        nc.tensor.matmul(bb_ps[:], ones_r[:], base1[:], start=True, stop=True)
        baseb = ppool.tile([128, 1], F32)
        nc.scalar.copy(baseb[:], bb_ps[:])

        rel = ppool.tile([128, PPP], F32)
        nc.vector.tensor_scalar(rel[:], flat[:], baseb[:, 0:1], None, ALU.subtract)
        hits = ppool.tile([128, PROBE * PROBE], F32)
        scratch = ppool.tile([128, PPP], F32)
        for i in range(PROBE * PROBE):
            off = float((i // PROBE) * 200 + (i % PROBE))
            nc.vector.tensor_scalar(scratch[:], rel[:], off, 0.0,
                                    ALU.is_equal, ALU.add,
                                    accum_out=hits[:, i:i + 1])
        cnt_ps = psmall.tile([1, PROBE * PROBE], F32, tag="aux")
        nc.tensor.matmul(cnt_ps[:], ones_c[:], hits[:], start=True, stop=True)
        occ1 = ppool.tile([1, PROBE * PROBE], F32)
        nc.vector.tensor_scalar(occ1[:], cnt_ps[:], 0.0, None, ALU.is_gt)
        ob_ps = psmall.tile([128, PROBE * PROBE], F32, tag="aux")
        nc.tensor.matmul(ob_ps[:], ones_r[:], occ1[:], start=True, stop=True)
        occb = ppool.tile([128, PROBE * PROBE], F32)
        nc.scalar.copy(occb[:], ob_ps[:])

        cell_i = ppool.tile([128, BEV_COLS], I32)
        nc.gpsimd.iota(cell_i[:], pattern=[[1, BEV_COLS]], base=0,
                       channel_multiplier=BEV_COLS)
        cell_f = ppool.tile([128, BEV_COLS], F32)
        nc.vector.tensor_copy(cell_f[:], cell_i[:])
        relc = ppool.tile([128, BEV_COLS], F32)
        nc.vector.tensor_scalar(relc[:], cell_f[:], baseb[:, 0:1], None,
                                ALU.subtract)
        bev = ppool.tile([128, BEV_COLS], F32)
        nc.vector.memset(bev[:], 0.0)
        mk = ppool.tile([128, BEV_COLS], F32)
        for i in range(PROBE * PROBE):
            off = float((i // PROBE) * 200 + (i % PROBE))
            nc.vector.tensor_scalar(mk[:], relc[:], off, None, ALU.is_equal)
            nc.vector.scalar_tensor_tensor(bev[:], mk[:], occb[:, i:i + 1],
                                           bev[:], ALU.mult, ALU.add)
        nc.sync.dma_start(bev_o, bev[:])

        # ---------------- BN + conv2 + softmax ----------------
        ctx2.close()
        spool = ctx.enter_context(tc.tile_pool(name="smax", bufs=1))
        gstats = hpool.tile([128, 2], F32)
        nc.sync.dma_start(gstats[:], cc_out.ap())

        bn_sb = cpool.tile([128, 3], F32)
        nc.sync.dma_start(bn_sb[:], bnp)
        mu = hpool.tile([128, 1], F32)
        nc.vector.tensor_scalar(mu[:], gstats[:, 0:1], 1.0 / TOTAL_POS, None, ALU.mult)
        var = hpool.tile([128, 1], F32)
        nc.vector.tensor_scalar(var[:], gstats[:, 1:2], 1.0 / TOTAL_POS, None, ALU.mult)
        mu2 = hpool.tile([128, 1], F32)
        nc.vector.tensor_tensor(mu2[:], mu[:], mu[:], ALU.mult)
        nc.vector.tensor_tensor(var[:], var[:], mu2[:], ALU.subtract)
        sd = hpool.tile([128, 1], F32)
        eps_t = hpool.tile([128, 1], F32)
        nc.vector.memset(eps_t[:], float(EPS))
        nc.scalar.activation(sd[:], var[:], ACTF.Sqrt, bias=eps_t[:])
        rinv = hpool.tile([128, 1], F32)
        nc.vector.reciprocal(rinv[:], sd[:])
        scale = hpool.tile([128, 1], F32)
        nc.vector.tensor_tensor(scale[:], rinv[:], bn_sb[:, 1:2], ALU.mult)
        bias_f = hpool.tile([128, 1], F32)
        nc.vector.tensor_tensor(bias_f[:], mu[:], bn_sb[:, 0:1], ALU.add)
        nc.vector.tensor_tensor(bias_f[:], bias_f[:], scale[:], ALU.mult)
        nc.vector.tensor_tensor(bias_f[:], bn_sb[:, 2:3], bias_f[:], ALU.subtract)

        h_relu = hpool.tile([128, QTOT], F32R)
        nc.scalar.activation(h_relu[:], h_raw[:], ACTF.Relu,
                             bias=bias_f[:], scale=scale[:])

        w2_sb = cpool.tile([128, 64], F32R)
        nc.sync.dma_start(w2_sb[:], w2t)
        b2_sb = cpool.tile([64, 2], F32)
        nc.sync.dma_start(b2_sb[:], b2b)
        bins_r = cpool.tile([64, 1], F32R)
        nc.vector.tensor_copy(bins_r[:], b2_sb[:, 1:2])

        exp_t = spool.tile([64, QTOT], F32R)
        den = spool.tile([1, QTOT], F32R)
        num = spool.tile([1, QTOT], F32)
        for c in range(NCHUNK):
            q0 = c * 512
            n = min(512, QTOT - q0)
            ps2 = psum.tile([64, 512], F32, tag="big")
            nc.tensor.matmul(ps2[:, 0:n], w2_sb[:], h_relu[:, q0:q0 + n],
                             start=True, stop=True)
            nc.scalar.activation(exp_t[:, q0:q0 + n], ps2[:, 0:n], ACTF.Exp,
                                 bias=b2_sb[:, 0:1], scale=1.0)
            psd = psmall.tile([1, 512], F32, tag="dn")
            nc.tensor.matmul(psd[:, 0:n], ones64r[:], exp_t[:, q0:q0 + n],
                             start=True, stop=True)
            nc.scalar.copy(den[:, q0:q0 + n], psd[:, 0:n])
            psn = psmall.tile([1, 512], F32, tag="dn")
            nc.tensor.matmul(psn[:, 0:n], bins_r[:], exp_t[:, q0:q0 + n],
                             start=True, stop=True)
            nc.scalar.copy(num[:, q0:q0 + n], psn[:, 0:n])

        rden = spool.tile([1, QTOT], F32)
        nc.vector.reciprocal(rden[:], den[:].bitcast(F32))
        rden_r = spool.tile([1, QTOT], F32R)
        nc.scalar.copy(rden_r[:], rden[:])
        ed1 = num
        nc.vector.tensor_tensor(ed1[:], num[:], rden[:], ALU.mult)
        nc.sync.dma_start(
            ed_o.rearrange("one (h w) -> one h w", h=HW_ROWS),
            ed1[:].rearrange("one (h w) -> one h w", h=HW_ROWS)[:, :, 0:FW])

        probs = spool.tile([64, QTOT], F32)
        for c in range(NCHUNK):
            q0 = c * 512
            n = min(512, QTOT - q0)
            psr = psum.tile([64, 512], F32, tag="big")
            nc.tensor.matmul(psr[:, 0:n], onesrow_r[:], rden_r[:, q0:q0 + n],
                             start=True, stop=True)
            nc.vector.tensor_tensor(probs[:, q0:q0 + n],
                                    exp_t[:, q0:q0 + n].bitcast(F32),
                                    psr[:, 0:n], ALU.mult)
        nc.sync.dma_start(
            dd_o.rearrange("d (h w) -> d h w", h=HW_ROWS),
            probs[:].rearrange("d (h w) -> d h w", h=HW_ROWS)[:, :, 0:FW])

    nc.compile()
    return nc


_NC_CACHE = None


def kernel(camera_features, pixels_uv, K_inv, W1, b1, gamma, beta, W2, b2,
           depth_bins):
    global _NC_CACHE
    if _NC_CACHE is None:
        _NC_CACHE = _build()
    nc = _NC_CACHE

    camera_features = np.asarray(camera_features, dtype=np.float32)
    pixels_uv = np.ascontiguousarray(np.asarray(pixels_uv, dtype=np.float32))
    K_inv = np.asarray(K_inv, dtype=np.float32)
    W1 = np.asarray(W1, dtype=np.float32)
    b1 = np.asarray(b1, dtype=np.float32)
    gamma = np.asarray(gamma, dtype=np.float32)
    beta = np.asarray(beta, dtype=np.float32)
    W2 = np.asarray(W2, dtype=np.float32)
    b2 = np.asarray(b2, dtype=np.float32)
    depth_bins = np.asarray(depth_bins, dtype=np.float32)

    # host-side layout prep (pure data movement)
    w1t = np.empty((18, 128, 128), np.float32)
    for ky in range(3):
        for kx in range(3):
            for kh in range(2):
                # [ci, co] for tap (ky,kx), K-half kh
                w1t[(ky * 3 + kx) * 2 + kh] = \
                    W1[:, kh * 128:(kh + 1) * 128, ky, kx].T
    xp = np.zeros((B, C_IN, FH + 2, WPAD), np.float32)
    xp[:, :, 1:FH + 1, 1:FW + 1] = camera_features
    w2t = np.ascontiguousarray(W2[:, :, 0, 0].T)
    bnp = np.ascontiguousarray(np.stack([b1, gamma, beta], axis=1))
    b2bins = np.ascontiguousarray(np.stack([b2, depth_bins], axis=1))

    in_maps = []
    for c in range(N_CORES):
        b = c // 2
        half = c % 2
        r0 = half * HW_ROWS
        uv_sl = pixels_uv[b, half * PPC:(half + 1) * PPC]
        uv_sh = np.concatenate([uv_sl, uv_sl[:PPC_PAD - PPC]], axis=0)
        in_maps.append({
            "xpad": np.ascontiguousarray(
                xp[b, :, r0:r0 + 34, :]).reshape(C_IN, 34 * WPAD),
            "uv": np.ascontiguousarray(uv_sh),
            "kinv": K_inv[b].reshape(1, 9).copy(),
            "w1t": w1t,
            "w2t": w2t,
            "bnp": bnp,
            "b2b": b2bins,
        })

    trace = bool(getattr(kernel, "_trace", False))
    res = run_bass_kernel_spmd(nc, in_maps, core_ids=list(range(N_CORES)),
                               trace=trace)
    kernel._last_exec_ns = res.exec_time_ns
    kernel._last_results = res

    dd = np.empty((B, D, FH, FW), np.float32)
    ed = np.empty((B, FH, FW), np.float32)
    pts = np.empty((B, N_PTS, 3), np.float32)
    bev = np.empty((B, 200, 200), np.float32)
    fallback = False
    for c in range(N_CORES):
        b = c // 2
        half = c % 2
        r0 = half * HW_ROWS
        r = res.results[c]
        dd[b, :, r0:r0 + HW_ROWS, :] = r["dd"].reshape(D, HW_ROWS, FW)
        ed[b, r0:r0 + HW_ROWS, :] = r["ed"].reshape(HW_ROWS, FW)
        pts[b, half * PPC:(half + 1) * PPC] = r["pts"][:PPC]
        g = r["bev"].ravel()[:40000].reshape(200, 200)
        flg = r["flg"].ravel()  # [-minx, maxx, -miny, maxy]
        if (flg[1] + flg[0] > PROBE - 1) or (flg[3] + flg[2] > PROBE - 1):
            fallback = True
        if half == 0:
            bev[b] = g
        else:
            np.maximum(bev[b], g, out=bev[b])

    if fallback:
        # exact host fallback (never taken for the target input distribution)
        for b in range(B):
            gx = np.clip(((pts[b, :, 0] + 50.0) / 0.5).astype(np.int32), 0, 199)
            gy = np.clip(((pts[b, :, 1] + 50.0) / 0.5).astype(np.int32), 0, 199)
            grid = np.zeros(40000, np.float32)
            np.add.at(grid, gy * 200 + gx, 1.0)
            bev[b] = np.clip(grid, 0.0, 1.0).reshape(200, 200)

    return bev, dd, ed, pts


# revision 20
# speedup vs baseline: 1.1605x; 1.1488x over previous
"""CameraOnlyBEV Trainium2 Bass kernel (8 NeuronCores, data-parallel over B x H).

Sharding: core c handles batch b = c//2, row-half h = c%2 (rows h*32..h*32+32 of
the 64-row feature map) for conv/BN/softmax, and points [h*75000,(h+1)*75000) of
batch b for lift/splat.

- conv1 (3x3, 256->128) = 18 accumulating fp32r matmuls per 512-position chunk
  (9 taps x 2 K-halves) over a zero-padded width-178 layout.
- BN training-mode batch stats are global over (B,H,W): per-core partial
  (sum, sum_sq) + one 1KB AllReduce across the 8 cores.
- depth = mean(softmax).mean() == 1/64 exactly (softmax sums to 1), so the
  point pipeline decouples from the conv pipeline and runs during the
  collective.
- BEV splat: exact occupancy semantics (clip(scatter_add(1),0,1)). All points
  of a shard land in a tiny cell bbox (1/64 depth scale maps everything near
  grid center); the kernel computes the exact bbox min and probes a 3x3 cell
  window with compare+count. A flag output lets the host fall back to an exact
  numpy splat if the bbox ever exceeds the window (never for this input
  distribution), so the kernel is correct for any input.
"""
import numpy as np
from contextlib import ExitStack

import concourse.bass as bass
import concourse.tile as tile
from concourse import bacc, mybir
from concourse.bass_utils import run_bass_kernel_spmd

F32 = mybir.dt.float32
F32R = mybir.dt.float32r
I32 = mybir.dt.int32
ALU = mybir.AluOpType
ACTF = mybir.ActivationFunctionType
AXL = mybir.AxisListType

N_CORES = 8
B, C_IN, FH, FW = 4, 256, 64, 176
N_PTS = 150000
D = 64
HW_ROWS = 32
WPAD = 178
QTOT = HW_ROWS * WPAD          # 5696
NCHUNK = (QTOT + 511) // 512   # 12
TOTAL_POS = B * FH * FW        # 45056
PPC = N_PTS // 2               # 75000 points per core
PPP = 587                      # points per partition (587*128 = 75136)
PPC_PAD = PPP * 128
EPS = 1e-5
DEPTH = float(np.float32(1.0 / 64.0))
PROBE = 3
BEV_COLS = 313                 # 128*313 = 40064 >= 40000


def _build():
    nc = bacc.Bacc("TRN2", target_bir_lowering=False, debug=False,
                   num_devices=N_CORES)

    xpad = nc.dram_tensor("xpad", [C_IN, 34 * WPAD], F32R, kind="ExternalInput").ap()
    uv = nc.dram_tensor("uv", [PPC_PAD, 2], F32, kind="ExternalInput").ap()
    kinv = nc.dram_tensor("kinv", [1, 9], F32, kind="ExternalInput").ap()
    w1t = nc.dram_tensor("w1t", [18, 128, 128], F32R, kind="ExternalInput").ap()
    w2t = nc.dram_tensor("w2t", [128, 64], F32R, kind="ExternalInput").ap()
    bnp = nc.dram_tensor("bnp", [128, 3], F32, kind="ExternalInput").ap()
    b2b = nc.dram_tensor("b2b", [64, 2], F32, kind="ExternalInput").ap()

    dd_o = nc.dram_tensor("dd", [64, HW_ROWS * FW], F32, kind="ExternalOutput").ap()
    ed_o = nc.dram_tensor("ed", [1, HW_ROWS * FW], F32, kind="ExternalOutput").ap()
    pts_o = nc.dram_tensor("pts", [PPC_PAD, 3], F32, kind="ExternalOutput").ap()
    bev_o = nc.dram_tensor("bev", [128, BEV_COLS], F32, kind="ExternalOutput").ap()
    flg_o = nc.dram_tensor("flg", [1, 4], F32, kind="ExternalOutput").ap()

    cc_in = nc.dram_tensor("cc_in", [128, 2], F32)
    cc_out = nc.dram_tensor("cc_out", [128, 2], F32, addr_space="Shared")

    with tile.TileContext(nc) as tc, ExitStack() as ctx:
        cpool = ctx.enter_context(tc.tile_pool(name="const", bufs=1))
        psum = ctx.enter_context(tc.tile_pool(name="psum", bufs=2, space="PSUM"))
        psmall = ctx.enter_context(tc.tile_pool(name="psum_s", bufs=2, space="PSUM"))
        wpool = ctx.enter_context(tc.tile_pool(name="w", bufs=1))
        hpool = ctx.enter_context(tc.tile_pool(name="h", bufs=1))
        ctx2 = ctx.enter_context(ExitStack())

        # ---------------- constants ----------------
        ones_r = cpool.tile([1, 128], F32)
        nc.vector.memset(ones_r[:], 1.0)
        ones_c = cpool.tile([128, 1], F32)
        nc.vector.memset(ones_c[:], 1.0)
        ones64f = cpool.tile([64, 1], F32)
        nc.vector.memset(ones64f[:], 1.0)
        ones64r = cpool.tile([64, 1], F32R)
        nc.vector.tensor_copy(ones64r[:], ones64f[:])
        onesrow_f = cpool.tile([1, 64], F32)
        nc.vector.memset(onesrow_f[:], 1.0)
        onesrow_r = cpool.tile([1, 64], F32R)
        nc.vector.tensor_copy(onesrow_r[:], onesrow_f[:])
        ident = cpool.tile([128, 128], F32)
        nc.vector.memset(ident[:], 1.0)
        nc.gpsimd.affine_select(ident[:], ident[:], pattern=[[-1, 128]],
                                compare_op=ALU.is_equal, fill=0.0,
                                base=0, channel_multiplier=1)

        # ---------------- conv1 ----------------
        ppool = ctx2.enter_context(tc.tile_pool(name="pts", bufs=1))
        xpool = ctx2.enter_context(tc.tile_pool(name="x", bufs=1))
        wts = wpool.tile([128, 18, 128], F32R)
        nc.sync.dma_start(wts[:], w1t.rearrange("t a b -> a t b"))

        xk = []
        for kh in range(2):
            xt = xpool.tile([128, 34 * WPAD + 2], F32R, tag=f"xk{kh}")
            zr2 = xpool.tile([128, 2], F32, tag="zr2")
            nc.vector.memset(zr2[:], 0.0)
            nc.vector.tensor_copy(xt[:, 34 * WPAD:], zr2[:])
            nc.sync.dma_start(xt[:, 0:34 * WPAD], xpad[kh * 128:(kh + 1) * 128, :])
            xk.append(xt)

        h_raw = hpool.tile([128, QTOT], F32)
        zero128 = cpool.tile([128, 1], F32)
        nc.vector.memset(zero128[:], 0.0)
        sumh_c = hpool.tile([128, NCHUNK], F32)
        sumq_c = hpool.tile([128, NCHUNK], F32)
        sq_scr = hpool.tile([128, 512], F32)

        for c in range(NCHUNK):
            q0 = c * 512
            n = min(512, QTOT - q0)
            ps = psum.tile([128, 512], F32, tag="big")
            k = 0
            for dy in range(3):
                for dx in range(3):
                    off = dy * WPAD + dx
                    for kh in range(2):
                        nc.tensor.matmul(
                            ps[:, 0:n],
                            wts[:, (dy * 3 + dx) * 2 + kh, :],
                            xk[kh][:, q0 + off:q0 + off + n],
                            start=(k == 0), stop=(k == 17),
                        )
                        k += 1
            nc.scalar.activation(h_raw[:, q0:q0 + n], ps[:, 0:n], ACTF.Copy,
                                 accum_out=sumh_c[:, c:c + 1])
            nc.scalar.activation(sq_scr[:, 0:n], ps[:, 0:n], ACTF.Square,
                                 bias=zero128[:], accum_out=sumq_c[:, c:c + 1])

        # stats correction for the 2 garbage cols per padded row
        garb = h_raw[:].rearrange("p (h w) -> p h w", h=HW_ROWS)[:, :, FW:WPAD]
        gsum = hpool.tile([128, 1], F32)
        nc.vector.tensor_reduce(gsum[:], garb, axis=AXL.XY, op=ALU.add)
        gsq_scr = hpool.tile([128, 64], F32)
        gsq = hpool.tile([128, 1], F32)
        nc.scalar.activation(gsq_scr[:], garb, ACTF.Square, bias=zero128[:],
                             accum_out=gsq[:])

        stats = hpool.tile([128, 2], F32)
        nc.vector.tensor_reduce(stats[:, 0:1], sumh_c[:], axis=AXL.X, op=ALU.add)
        nc.vector.tensor_reduce(stats[:, 1:2], sumq_c[:], axis=AXL.X, op=ALU.add)
        nc.vector.tensor_tensor(stats[:, 0:1], stats[:, 0:1], gsum[:], ALU.subtract)
        nc.vector.tensor_tensor(stats[:, 1:2], stats[:, 1:2], gsq[:], ALU.subtract)

        nc.sync.dma_start(cc_in.ap(), stats[:])
        nc.gpsimd.collective_compute(
            "AllReduce", ALU.add,
            replica_groups=[list(range(N_CORES))],
            ins=[cc_in.ap()], outs=[cc_out.ap()],
        )

        # ---------------- point pipeline (overlaps the collective) ---------
        kin_sb = cpool.tile([1, 9], F32)
        nc.sync.dma_start(kin_sb[:], kinv)
        kb_ps = psmall.tile([128, 9], F32, tag="aux")
        nc.tensor.matmul(kb_ps[:], ones_r[:], kin_sb[:], start=True, stop=True)
        kb = cpool.tile([128, 9], F32)
        nc.scalar.copy(kb[:], kb_ps[:])

        uv_t = ppool.tile([128, PPP, 2], F32)
        nc.sync.dma_start(uv_t[:], uv.rearrange("(p c) k -> p c k", p=128))
        u = uv_t[:, :, 0]
        v = uv_t[:, :, 1]

        xyz = ppool.tile([128, PPP, 3], F32)
        t0 = ppool.tile([128, PPP], F32)
        gxf = ppool.tile([128, PPP], F32)
        gyf = ppool.tile([128, PPP], F32)
        flat = ppool.tile([128, PPP], F32)

        # z (j=2): plain two-step (loose tolerance)
        nc.vector.tensor_scalar(t0[:], u, kb[:, 6:7], None, ALU.mult)
        nc.vector.scalar_tensor_tensor(t0[:], v, kb[:, 7:8], t0[:],
                                       ALU.mult, ALU.add)
        nc.vector.tensor_scalar(xyz[:, :, 2], t0[:], kb[:, 8:9], DEPTH,
                                ALU.add, ALU.mult)

        # x, y (j=0,1): r2 = fma(v, K[j,1], u*K[j,0]) emulated via Dekker so
        # the cell assignment matches the reference's XLA fma chain bitwise.
        SPLITC = 4097.0
        bh = ppool.tile([128, PPP], F32)
        bl = ppool.tile([128, PPP], F32)
        nc.vector.tensor_scalar(bh[:], v, SPLITC, None, ALU.mult)     # tb
        nc.vector.tensor_tensor(bl[:], bh[:], v, ALU.subtract)        # tb - v
        nc.vector.tensor_tensor(bh[:], bh[:], bl[:], ALU.subtract)    # bh
        nc.vector.tensor_tensor(bl[:], v, bh[:], ALU.subtract)        # bl
        ksp = ppool.tile([128, 6], F32)  # per-j: [ah, al, ta] x2
        for j in range(2):
            a = kb[:, 3 * j + 1:3 * j + 2]
            ta = ksp[:, 3 * j + 2:3 * j + 3]
            ah = ksp[:, 3 * j:3 * j + 1]
            al = ksp[:, 3 * j + 1:3 * j + 2]
            nc.vector.tensor_scalar(ta, a, SPLITC, None, ALU.mult)
            nc.vector.tensor_tensor(ah, ta, a, ALU.subtract)
            nc.vector.tensor_tensor(ah, ta, ah, ALU.subtract)
            nc.vector.tensor_tensor(al, a, ah, ALU.subtract)
        e1 = ppool.tile([128, PPP], F32)
        e2 = ppool.tile([128, PPP], F32)
        pp = ppool.tile([128, PPP], F32)
        for j in range(2):
            a = kb[:, 3 * j + 1:3 * j + 2]
            ah = ksp[:, 3 * j:3 * j + 1]
            al = ksp[:, 3 * j + 1:3 * j + 2]
            nc.vector.tensor_scalar(pp[:], v, a, None, ALU.mult)          # p
            nc.vector.scalar_tensor_tensor(e1[:], bh[:], ah, pp[:],
                                           ALU.mult, ALU.subtract)        # bh*ah - p
            nc.vector.scalar_tensor_tensor(e2[:], bh[:], al, e1[:],
                                           ALU.mult, ALU.add)
            nc.vector.scalar_tensor_tensor(e1[:], bl[:], ah, e2[:],
                                           ALU.mult, ALU.add)
            nc.vector.scalar_tensor_tensor(e1[:], bl[:], al, e1[:],
                                           ALU.mult, ALU.add)             # err
            nc.vector.tensor_scalar(t0[:], u, kb[:, 3 * j:3 * j + 1],
                                    None, ALU.mult)                       # r1
            # TwoSum(r1, p)
            s_ = gxf if j == 0 else gyf  # reuse as scratch for s
            nc.vector.tensor_tensor(s_[:], t0[:], pp[:], ALU.add)         # s
            nc.vector.tensor_tensor(e2[:], s_[:], t0[:], ALU.subtract)    # bb
            nc.vector.tensor_tensor(flat[:], s_[:], e2[:], ALU.subtract)  # s-bb
            nc.vector.tensor_tensor(flat[:], t0[:], flat[:], ALU.subtract)  # t-(s-bb)
            nc.vector.tensor_tensor(e2[:], pp[:], e2[:], ALU.subtract)    # p-bb
            nc.vector.tensor_tensor(e2[:], flat[:], e2[:], ALU.add)       # ee
            nc.vector.tensor_tensor(e1[:], e1[:], e2[:], ALU.add)         # err+ee
            nc.vector.tensor_tensor(s_[:], s_[:], e1[:], ALU.add)         # r2
            nc.vector.tensor_scalar(xyz[:, :, j], s_[:],
                                    kb[:, 3 * j + 2:3 * j + 3], DEPTH,
                                    ALU.add, ALU.mult)
        nc.sync.dma_start(pts_o.rearrange("(p c) k -> p (c k)", p=128),
                          xyz[:].rearrange("p c k -> p (c k)"))

        vi = ppool.tile([128, PPP], I32)
        cf = ppool.tile([128, PPP], F32)
        for src_j, gout in ((0, gxf), (1, gyf)):
            nc.vector.tensor_scalar(t0[:], xyz[:, :, src_j], 50.0, 2.0,
                                    ALU.add, ALU.mult)
            nc.vector.tensor_scalar(t0[:], t0[:], 0.0, 199.0, ALU.max, ALU.min)
            nc.vector.tensor_copy(vi[:], t0[:])
            nc.vector.tensor_copy(cf[:], vi[:])
            nc.vector.tensor_tensor(gout[:], cf[:], t0[:], ALU.is_gt)
            nc.vector.tensor_tensor(gout[:], cf[:], gout[:], ALU.subtract)
        nc.vector.scalar_tensor_tensor(flat[:], gyf[:], 200.0, gxf[:],
                                       ALU.mult, ALU.add)

        st = ppool.tile([128, 4], F32)
        stn = ppool.tile([128, 2], F32)
        nc.vector.tensor_reduce(stn[:, 0:1], gxf[:], axis=AXL.X, op=ALU.min)
        nc.vector.tensor_reduce(st[:, 1:2], gxf[:], axis=AXL.X, op=ALU.max)
        nc.vector.tensor_reduce(stn[:, 1:2], gyf[:], axis=AXL.X, op=ALU.min)
        nc.vector.tensor_reduce(st[:, 3:4], gyf[:], axis=AXL.X, op=ALU.max)
        nc.vector.tensor_scalar(st[:, 0:1], stn[:, 0:1], -1.0, None, ALU.mult)
        nc.vector.tensor_scalar(st[:, 2:3], stn[:, 1:2], -1.0, None, ALU.mult)
        stt_ps = psmall.tile([4, 128], F32, tag="aux")
        nc.tensor.transpose(stt_ps[:], st[:], ident[:])
        gst = ppool.tile([4, 1], F32)
        nc.vector.tensor_reduce(gst[:], stt_ps[:], axis=AXL.X, op=ALU.max)
        g_ps = psmall.tile([1, 4], F32, tag="aux")
        nc.tensor.matmul(g_ps[:], gst[:, 0:1], ident[0:4, 0:4],
                         start=True, stop=True)
        g_row = ppool.tile([1, 4], F32)
        nc.scalar.copy(g_row[:], g_ps[:])
        nc.sync.dma_start(flg_o, g_row[:])

        base1 = ppool.tile([1, 1], F32)
        nc.vector.scalar_tensor_tensor(base1[:], g_row[:, 2:3], 200.0,
                                       g_row[:, 0:1], ALU.mult, ALU.add)
        nc.vector.tensor_scalar(base1[:], base1[:], -1.0, None, ALU.mult)
        bb_ps = psmall.tile([128, 1], F32, tag="aux")
        nc.tensor.matmul(bb_ps[:], ones_r[:], base1[:], start=True, stop=True)
        baseb = ppool.tile([128, 1], F32)
        nc.scalar.copy(baseb[:], bb_ps[:])

        rel = ppool.tile([128, PPP], F32)
        nc.vector.tensor_scalar(rel[:], flat[:], baseb[:, 0:1], None, ALU.subtract)
        hits = ppool.tile([128, PROBE * PROBE], F32)
        scratch = ppool.tile([128, PPP], F32)
        for i in range(PROBE * PROBE):
            off = float((i // PROBE) * 200 + (i % PROBE))
            nc.vector.tensor_scalar(scratch[:], rel[:], off, 0.0,
                                    ALU.is_equal, ALU.add,
                                    accum_out=hits[:, i:i + 1])
        cnt_ps = psmall.tile([1, PROBE * PROBE], F32, tag="aux")
        nc.tensor.matmul(cnt_ps[:], ones_c[:], hits[:], start=True, stop=True)
        occ1 = ppool.tile([1, PROBE * PROBE], F32)
        nc.vector.tensor_scalar(occ1[:], cnt_ps[:], 0.0, None, ALU.is_gt)
        ob_ps = psmall.tile([128, PROBE * PROBE], F32, tag="aux")
        nc.tensor.matmul(ob_ps[:], ones_r[:], occ1[:], start=True, stop=True)
        occb = ppool.tile([128, PROBE * PROBE], F32)
        nc.scalar.copy(occb[:], ob_ps[:])

        cell_i = ppool.tile([128, BEV_COLS], I32)
        nc.gpsimd.iota(cell_i[:], pattern=[[1, BEV_COLS]], base=0,
                       channel_multiplier=BEV_COLS)
        cell_f = ppool.tile([128, BEV_COLS], F32)
        nc.vector.tensor_copy(cell_f[:], cell_i[:])
        relc = ppool.tile([128, BEV_COLS], F32)
        nc.vector.tensor_scalar(relc[:], cell_f[:], baseb[:, 0:1], None,
                                ALU.subtract)
        bev = ppool.tile([128, BEV_COLS], F32)
        nc.vector.memset(bev[:], 0.0)
        mk = ppool.tile([128, BEV_COLS], F32)
        for i in range(PROBE * PROBE):
            off = float((i // PROBE) * 200 + (i % PROBE))
            nc.vector.tensor_scalar(mk[:], relc[:], off, None, ALU.is_equal)
            nc.vector.scalar_tensor_tensor(bev[:], mk[:], occb[:, i:i + 1],
                                           bev[:], ALU.mult, ALU.add)
        nc.sync.dma_start(bev_o, bev[:])

        # ---------------- BN + conv2 + softmax ----------------
        ctx2.close()
        spool = ctx.enter_context(tc.tile_pool(name="smax", bufs=1))
        gstats = hpool.tile([128, 2], F32)
        nc.sync.dma_start(gstats[:], cc_out.ap())

        bn_sb = cpool.tile([128, 3], F32)
        nc.sync.dma_start(bn_sb[:], bnp)
        mu = hpool.tile([128, 1], F32)
        nc.vector.tensor_scalar(mu[:], gstats[:, 0:1], 1.0 / TOTAL_POS, None, ALU.mult)
        var = hpool.tile([128, 1], F32)
        nc.vector.tensor_scalar(var[:], gstats[:, 1:2], 1.0 / TOTAL_POS, None, ALU.mult)
        mu2 = hpool.tile([128, 1], F32)
        nc.vector.tensor_tensor(mu2[:], mu[:], mu[:], ALU.mult)
        nc.vector.tensor_tensor(var[:], var[:], mu2[:], ALU.subtract)
        sd = hpool.tile([128, 1], F32)
        eps_t = hpool.tile([128, 1], F32)
        nc.vector.memset(eps_t[:], float(EPS))
        nc.scalar.activation(sd[:], var[:], ACTF.Sqrt, bias=eps_t[:])
        rinv = hpool.tile([128, 1], F32)
        nc.vector.reciprocal(rinv[:], sd[:])
        scale = hpool.tile([128, 1], F32)
        nc.vector.tensor_tensor(scale[:], rinv[:], bn_sb[:, 1:2], ALU.mult)
        bias_f = hpool.tile([128, 1], F32)
        nc.vector.tensor_tensor(bias_f[:], mu[:], bn_sb[:, 0:1], ALU.add)
        nc.vector.tensor_tensor(bias_f[:], bias_f[:], scale[:], ALU.mult)
        nc.vector.tensor_tensor(bias_f[:], bn_sb[:, 2:3], bias_f[:], ALU.subtract)

        h_relu = hpool.tile([128, QTOT], F32R)
        nc.scalar.activation(h_relu[:], h_raw[:], ACTF.Relu,
                             bias=bias_f[:], scale=scale[:])

        w2_sb = cpool.tile([128, 64], F32R)
        nc.sync.dma_start(w2_sb[:], w2t)
        b2_sb = cpool.tile([64, 2], F32)
        nc.sync.dma_start(b2_sb[:], b2b)
        bins_r = cpool.tile([64, 1], F32R)
        nc.vector.tensor_copy(bins_r[:], b2_sb[:, 1:2])

        exp_t = spool.tile([64, QTOT], F32R)
        den = spool.tile([1, QTOT], F32R)
        num = spool.tile([1, QTOT], F32)
        rden = spool.tile([1, QTOT], F32)
        rden_r = spool.tile([1, QTOT], F32R)
        for c in range(NCHUNK):
            q0 = c * 512
            n = min(512, QTOT - q0)
            ps2 = psum.tile([64, 512], F32, tag="big")
            nc.tensor.matmul(ps2[:, 0:n], w2_sb[:], h_relu[:, q0:q0 + n],
                             start=True, stop=True)
            nc.scalar.activation(exp_t[:, q0:q0 + n], ps2[:, 0:n], ACTF.Exp,
                                 bias=b2_sb[:, 0:1], scale=1.0)
            psd = psmall.tile([1, 512], F32, tag="dn")
            nc.tensor.matmul(psd[:, 0:n], ones64r[:], exp_t[:, q0:q0 + n],
                             start=True, stop=True)
            nc.scalar.copy(den[:, q0:q0 + n], psd[:, 0:n])
            psn = psmall.tile([1, 512], F32, tag="dn")
            nc.tensor.matmul(psn[:, 0:n], bins_r[:], exp_t[:, q0:q0 + n],
                             start=True, stop=True)
            nc.scalar.copy(num[:, q0:q0 + n], psn[:, 0:n])
            nc.vector.reciprocal(rden[:, q0:q0 + n], den[:, q0:q0 + n].bitcast(F32))
            nc.scalar.copy(rden_r[:, q0:q0 + n], rden[:, q0:q0 + n])

        ed1 = num
        nc.vector.tensor_tensor(ed1[:], num[:], rden[:], ALU.mult)
        nc.sync.dma_start(
            ed_o.rearrange("one (h w) -> one h w", h=HW_ROWS),
            ed1[:].rearrange("one (h w) -> one h w", h=HW_ROWS)[:, :, 0:FW])

        probs = spool.tile([64, QTOT], F32)
        for c in range(NCHUNK):
            q0 = c * 512
            n = min(512, QTOT - q0)
            psr = psum.tile([64, 512], F32, tag="big")
            nc.tensor.matmul(psr[:, 0:n], onesrow_r[:], rden_r[:, q0:q0 + n],
                             start=True, stop=True)
            nc.vector.tensor_tensor(probs[:, q0:q0 + n],
                                    exp_t[:, q0:q0 + n].bitcast(F32),
                                    psr[:, 0:n], ALU.mult)
        nc.sync.dma_start(
            dd_o.rearrange("d (h w) -> d h w", h=HW_ROWS),
            probs[:].rearrange("d (h w) -> d h w", h=HW_ROWS)[:, :, 0:FW])

    nc.compile()
    return nc


_NC_CACHE = None


def kernel(camera_features, pixels_uv, K_inv, W1, b1, gamma, beta, W2, b2,
           depth_bins):
    global _NC_CACHE
    if _NC_CACHE is None:
        _NC_CACHE = _build()
    nc = _NC_CACHE

    camera_features = np.asarray(camera_features, dtype=np.float32)
    pixels_uv = np.ascontiguousarray(np.asarray(pixels_uv, dtype=np.float32))
    K_inv = np.asarray(K_inv, dtype=np.float32)
    W1 = np.asarray(W1, dtype=np.float32)
    b1 = np.asarray(b1, dtype=np.float32)
    gamma = np.asarray(gamma, dtype=np.float32)
    beta = np.asarray(beta, dtype=np.float32)
    W2 = np.asarray(W2, dtype=np.float32)
    b2 = np.asarray(b2, dtype=np.float32)
    depth_bins = np.asarray(depth_bins, dtype=np.float32)

    # host-side layout prep (pure data movement)
    w1t = np.empty((18, 128, 128), np.float32)
    for ky in range(3):
        for kx in range(3):
            for kh in range(2):
                # [ci, co] for tap (ky,kx), K-half kh
                w1t[(ky * 3 + kx) * 2 + kh] = \
                    W1[:, kh * 128:(kh + 1) * 128, ky, kx].T
    xp = np.zeros((B, C_IN, FH + 2, WPAD), np.float32)
    xp[:, :, 1:FH + 1, 1:FW + 1] = camera_features
    w2t = np.ascontiguousarray(W2[:, :, 0, 0].T)
    bnp = np.ascontiguousarray(np.stack([b1, gamma, beta], axis=1))
    b2bins = np.ascontiguousarray(np.stack([b2, depth_bins], axis=1))

    in_maps = []
    for c in range(N_CORES):
        b = c // 2
        half = c % 2
        r0 = half * HW_ROWS
        uv_sl = pixels_uv[b, half * PPC:(half + 1) * PPC]
        uv_sh = np.concatenate([uv_sl, uv_sl[:PPC_PAD - PPC]], axis=0)
        in_maps.append({
            "xpad": np.ascontiguousarray(
                xp[b, :, r0:r0 + 34, :]).reshape(C_IN, 34 * WPAD),
            "uv": np.ascontiguousarray(uv_sh),
            "kinv": K_inv[b].reshape(1, 9).copy(),
            "w1t": w1t,
            "w2t": w2t,
            "bnp": bnp,
            "b2b": b2bins,
        })

    trace = bool(getattr(kernel, "_trace", False))
    res = run_bass_kernel_spmd(nc, in_maps, core_ids=list(range(N_CORES)),
                               trace=trace)
    kernel._last_exec_ns = res.exec_time_ns
    kernel._last_results = res

    dd = np.empty((B, D, FH, FW), np.float32)
    ed = np.empty((B, FH, FW), np.float32)
    pts = np.empty((B, N_PTS, 3), np.float32)
    bev = np.empty((B, 200, 200), np.float32)
    fallback = False
    for c in range(N_CORES):
        b = c // 2
        half = c % 2
        r0 = half * HW_ROWS
        r = res.results[c]
        dd[b, :, r0:r0 + HW_ROWS, :] = r["dd"].reshape(D, HW_ROWS, FW)
        ed[b, r0:r0 + HW_ROWS, :] = r["ed"].reshape(HW_ROWS, FW)
        pts[b, half * PPC:(half + 1) * PPC] = r["pts"][:PPC]
        g = r["bev"].ravel()[:40000].reshape(200, 200)
        flg = r["flg"].ravel()  # [-minx, maxx, -miny, maxy]
        if (flg[1] + flg[0] > PROBE - 1) or (flg[3] + flg[2] > PROBE - 1):
            fallback = True
        if half == 0:
            bev[b] = g
        else:
            np.maximum(bev[b], g, out=bev[b])

    if fallback:
        # exact host fallback (never taken for the target input distribution)
        for b in range(B):
            gx = np.clip(((pts[b, :, 0] + 50.0) / 0.5).astype(np.int32), 0, 199)
            gy = np.clip(((pts[b, :, 1] + 50.0) / 0.5).astype(np.int32), 0, 199)
            grid = np.zeros(40000, np.float32)
            np.add.at(grid, gy * 200 + gx, 1.0)
            bev[b] = np.clip(grid, 0.0, 1.0).reshape(200, 200)

    return bev, dd, ed, pts


# revision 23
# speedup vs baseline: 1.2370x; 1.0659x over previous
"""CameraOnlyBEV Trainium2 Bass kernel (8 NeuronCores, data-parallel over B x H).

Sharding: core c handles batch b = c//2, row-half h = c%2 (rows h*32..h*32+32 of
the 64-row feature map) for conv/BN/softmax, and points [h*75000,(h+1)*75000) of
batch b for lift/splat.

- conv1 (3x3, 256->128) = 18 accumulating fp32r matmuls per 512-position chunk
  (9 taps x 2 K-halves) over a zero-padded width-178 layout.
- BN training-mode batch stats are global over (B,H,W): per-core partial
  (sum, sum_sq) + one 1KB AllReduce across the 8 cores.
- depth = mean(softmax).mean() == 1/64 exactly (softmax sums to 1), so the
  point pipeline decouples from the conv pipeline and runs during the
  collective.
- BEV splat: exact occupancy semantics (clip(scatter_add(1),0,1)). All points
  of a shard land in a tiny cell bbox (1/64 depth scale maps everything near
  grid center); the kernel computes the exact bbox min and probes a 3x3 cell
  window with compare+count. A flag output lets the host fall back to an exact
  numpy splat if the bbox ever exceeds the window (never for this input
  distribution), so the kernel is correct for any input.
"""
import numpy as np
from contextlib import ExitStack

import concourse.bass as bass
import concourse.tile as tile
from concourse import bacc, mybir
from concourse.bass_utils import run_bass_kernel_spmd

F32 = mybir.dt.float32
F32R = mybir.dt.float32r
I32 = mybir.dt.int32
ALU = mybir.AluOpType
ACTF = mybir.ActivationFunctionType
AXL = mybir.AxisListType

N_CORES = 8
B, C_IN, FH, FW = 4, 256, 64, 176
N_PTS = 150000
D = 64
HW_ROWS = 32
WPAD = 178
QTOT = HW_ROWS * WPAD          # 5696
NCHUNK = (QTOT + 511) // 512   # 12
TOTAL_POS = B * FH * FW        # 45056
PPC = N_PTS // 2               # 75000 points per core
PPP = 587                      # points per partition (587*128 = 75136)
PPC_PAD = PPP * 128
EPS = 1e-5
DEPTH = float(np.float32(1.0 / 64.0))
PROBE = 3
BEV_COLS = 313                 # 128*313 = 40064 >= 40000


def _build():
    nc = bacc.Bacc("TRN2", target_bir_lowering=False, debug=False,
                   num_devices=N_CORES)

    xpad = nc.dram_tensor("xpad", [C_IN, 34 * WPAD], F32R, kind="ExternalInput").ap()
    uv = nc.dram_tensor("uv", [PPC_PAD, 2], F32, kind="ExternalInput").ap()
    kinv = nc.dram_tensor("kinv", [1, 9], F32, kind="ExternalInput").ap()
    w1t = nc.dram_tensor("w1t", [18, 128, 128], F32R, kind="ExternalInput").ap()
    w2t = nc.dram_tensor("w2t", [128, 64], F32R, kind="ExternalInput").ap()
    bnp = nc.dram_tensor("bnp", [128, 3], F32, kind="ExternalInput").ap()
    b2b = nc.dram_tensor("b2b", [64, 2], F32, kind="ExternalInput").ap()

    dd_o = nc.dram_tensor("dd", [64, HW_ROWS * FW], F32, kind="ExternalOutput").ap()
    ed_o = nc.dram_tensor("ed", [1, HW_ROWS * FW], F32, kind="ExternalOutput").ap()
    pts_o = nc.dram_tensor("pts", [PPC_PAD, 3], F32, kind="ExternalOutput").ap()
    bev_o = nc.dram_tensor("bev", [128, BEV_COLS], F32, kind="ExternalOutput").ap()
    flg_o = nc.dram_tensor("flg", [1, 4], F32, kind="ExternalOutput").ap()

    cc_in = nc.dram_tensor("cc_in", [128, 2], F32)
    cc_out = nc.dram_tensor("cc_out", [128, 2], F32, addr_space="Shared")

    with tile.TileContext(nc) as tc, ExitStack() as ctx:
        cpool = ctx.enter_context(tc.tile_pool(name="const", bufs=1))
        psum = ctx.enter_context(tc.tile_pool(name="psum", bufs=4, space="PSUM"))
        psmall = ctx.enter_context(tc.tile_pool(name="psum_s", bufs=2, space="PSUM"))
        wpool = ctx.enter_context(tc.tile_pool(name="w", bufs=1))
        hpool = ctx.enter_context(tc.tile_pool(name="h", bufs=1))
        ctx2 = ctx.enter_context(ExitStack())

        # ---------------- constants ----------------
        ones_r = cpool.tile([1, 128], F32)
        nc.vector.memset(ones_r[:], 1.0)
        ones_c = cpool.tile([128, 1], F32)
        nc.vector.memset(ones_c[:], 1.0)
        ones64f = cpool.tile([64, 1], F32)
        nc.vector.memset(ones64f[:], 1.0)
        ones64r = cpool.tile([64, 1], F32R)
        nc.vector.tensor_copy(ones64r[:], ones64f[:])
        onesrow_f = cpool.tile([1, 64], F32)
        nc.vector.memset(onesrow_f[:], 1.0)
        onesrow_r = cpool.tile([1, 64], F32R)
        nc.vector.tensor_copy(onesrow_r[:], onesrow_f[:])
        ident = cpool.tile([128, 128], F32)
        nc.vector.memset(ident[:], 1.0)
        nc.gpsimd.affine_select(ident[:], ident[:], pattern=[[-1, 128]],
                                compare_op=ALU.is_equal, fill=0.0,
                                base=0, channel_multiplier=1)

        # ---------------- conv1 ----------------
        ppool = ctx2.enter_context(tc.tile_pool(name="pts", bufs=1))
        xpool = ctx2.enter_context(tc.tile_pool(name="x", bufs=1))
        wts = wpool.tile([128, 18, 128], F32R)
        nc.sync.dma_start(wts[:], w1t.rearrange("t a b -> a t b"))

        xk = []
        for kh in range(2):
            xt = xpool.tile([128, 34 * WPAD + 2], F32R, tag=f"xk{kh}")
            zr2 = xpool.tile([128, 2], F32, tag="zr2")
            nc.vector.memset(zr2[:], 0.0)
            nc.vector.tensor_copy(xt[:, 34 * WPAD:], zr2[:])
            nc.sync.dma_start(xt[:, 0:34 * WPAD], xpad[kh * 128:(kh + 1) * 128, :])
            xk.append(xt)

        h_raw = hpool.tile([128, QTOT], F32)
        zero128 = cpool.tile([128, 1], F32)
        nc.vector.memset(zero128[:], 0.0)
        sumh_c = hpool.tile([128, NCHUNK], F32)
        sumq_c = hpool.tile([128, NCHUNK], F32)
        sq_scr = hpool.tile([128, 512], F32)

        for c in range(NCHUNK):
            q0 = c * 512
            n = min(512, QTOT - q0)
            ps = psum.tile([128, 512], F32, tag="big")
            k = 0
            for dy in range(3):
                for dx in range(3):
                    off = dy * WPAD + dx
                    for kh in range(2):
                        nc.tensor.matmul(
                            ps[:, 0:n],
                            wts[:, (dy * 3 + dx) * 2 + kh, :],
                            xk[kh][:, q0 + off:q0 + off + n],
                            start=(k == 0), stop=(k == 17),
                        )
                        k += 1
            nc.scalar.activation(h_raw[:, q0:q0 + n], ps[:, 0:n], ACTF.Copy,
                                 accum_out=sumh_c[:, c:c + 1])
            nc.scalar.activation(sq_scr[:, 0:n], ps[:, 0:n], ACTF.Square,
                                 bias=zero128[:], accum_out=sumq_c[:, c:c + 1])

        # stats correction for the 2 garbage cols per padded row
        garb = h_raw[:].rearrange("p (h w) -> p h w", h=HW_ROWS)[:, :, FW:WPAD]
        gsum = hpool.tile([128, 1], F32)
        nc.vector.tensor_reduce(gsum[:], garb, axis=AXL.XY, op=ALU.add)
        gsq_scr = hpool.tile([128, 64], F32)
        gsq = hpool.tile([128, 1], F32)
        nc.scalar.activation(gsq_scr[:], garb, ACTF.Square, bias=zero128[:],
                             accum_out=gsq[:])

        stats = hpool.tile([128, 2], F32)
        nc.vector.tensor_reduce(stats[:, 0:1], sumh_c[:], axis=AXL.X, op=ALU.add)
        nc.vector.tensor_reduce(stats[:, 1:2], sumq_c[:], axis=AXL.X, op=ALU.add)
        nc.vector.tensor_tensor(stats[:, 0:1], stats[:, 0:1], gsum[:], ALU.subtract)
        nc.vector.tensor_tensor(stats[:, 1:2], stats[:, 1:2], gsq[:], ALU.subtract)

        nc.sync.dma_start(cc_in.ap(), stats[:])
        nc.gpsimd.collective_compute(
            "AllReduce", ALU.add,
            replica_groups=[list(range(N_CORES))],
            ins=[cc_in.ap()], outs=[cc_out.ap()],
        )

        # ---------------- point pipeline (overlaps the collective) ---------
        kin_sb = cpool.tile([1, 9], F32)
        nc.sync.dma_start(kin_sb[:], kinv)
        kb_ps = psmall.tile([128, 9], F32, tag="aux")
        nc.tensor.matmul(kb_ps[:], ones_r[:], kin_sb[:], start=True, stop=True)
        kb = cpool.tile([128, 9], F32)
        nc.scalar.copy(kb[:], kb_ps[:])

        uv_t = ppool.tile([128, PPP, 2], F32)
        nc.sync.dma_start(uv_t[:], uv.rearrange("(p c) k -> p c k", p=128))
        u = uv_t[:, :, 0]
        v = uv_t[:, :, 1]

        xyz = ppool.tile([128, PPP, 3], F32)
        t0 = ppool.tile([128, PPP], F32)
        gxf = ppool.tile([128, PPP], F32)
        gyf = ppool.tile([128, PPP], F32)
        flat = ppool.tile([128, PPP], F32)

        # z (j=2): plain two-step (loose tolerance)
        nc.vector.tensor_scalar(t0[:], u, kb[:, 6:7], None, ALU.mult)
        nc.vector.scalar_tensor_tensor(t0[:], v, kb[:, 7:8], t0[:],
                                       ALU.mult, ALU.add)
        nc.vector.tensor_scalar(xyz[:, :, 2], t0[:], kb[:, 8:9], DEPTH,
                                ALU.add, ALU.mult)

        # x, y (j=0,1): r2 = fma(v, K[j,1], u*K[j,0]) emulated via Dekker so
        # the cell assignment matches the reference's XLA fma chain bitwise.
        SPLITC = 4097.0
        bh = ppool.tile([128, PPP], F32)
        bl = ppool.tile([128, PPP], F32)
        nc.vector.tensor_scalar(bh[:], v, SPLITC, None, ALU.mult)     # tb
        nc.vector.tensor_tensor(bl[:], bh[:], v, ALU.subtract)        # tb - v
        nc.vector.tensor_tensor(bh[:], bh[:], bl[:], ALU.subtract)    # bh
        nc.vector.tensor_tensor(bl[:], v, bh[:], ALU.subtract)        # bl
        ksp = ppool.tile([128, 6], F32)  # per-j: [ah, al, ta] x2
        for j in range(2):
            a = kb[:, 3 * j + 1:3 * j + 2]
            ta = ksp[:, 3 * j + 2:3 * j + 3]
            ah = ksp[:, 3 * j:3 * j + 1]
            al = ksp[:, 3 * j + 1:3 * j + 2]
            nc.vector.tensor_scalar(ta, a, SPLITC, None, ALU.mult)
            nc.vector.tensor_tensor(ah, ta, a, ALU.subtract)
            nc.vector.tensor_tensor(ah, ta, ah, ALU.subtract)
            nc.vector.tensor_tensor(al, a, ah, ALU.subtract)
        e1 = ppool.tile([128, PPP], F32)
        e2 = ppool.tile([128, PPP], F32)
        pp = ppool.tile([128, PPP], F32)
        for j in range(2):
            a = kb[:, 3 * j + 1:3 * j + 2]
            ah = ksp[:, 3 * j:3 * j + 1]
            al = ksp[:, 3 * j + 1:3 * j + 2]
            nc.vector.tensor_scalar(pp[:], v, a, None, ALU.mult)          # p
            nc.vector.scalar_tensor_tensor(e1[:], bh[:], ah, pp[:],
                                           ALU.mult, ALU.subtract)        # bh*ah - p
            nc.vector.scalar_tensor_tensor(e2[:], bh[:], al, e1[:],
                                           ALU.mult, ALU.add)
            nc.vector.scalar_tensor_tensor(e1[:], bl[:], ah, e2[:],
                                           ALU.mult, ALU.add)
            nc.vector.scalar_tensor_tensor(e1[:], bl[:], al, e1[:],
                                           ALU.mult, ALU.add)             # err
            nc.vector.tensor_scalar(t0[:], u, kb[:, 3 * j:3 * j + 1],
                                    None, ALU.mult)                       # r1
            # TwoSum(r1, p)
            s_ = gxf if j == 0 else gyf  # reuse as scratch for s
            nc.vector.tensor_tensor(s_[:], t0[:], pp[:], ALU.add)         # s
            nc.vector.tensor_tensor(e2[:], s_[:], t0[:], ALU.subtract)    # bb
            nc.vector.tensor_tensor(flat[:], s_[:], e2[:], ALU.subtract)  # s-bb
            nc.vector.tensor_tensor(flat[:], t0[:], flat[:], ALU.subtract)  # t-(s-bb)
            nc.vector.tensor_tensor(e2[:], pp[:], e2[:], ALU.subtract)    # p-bb
            nc.vector.tensor_tensor(e2[:], flat[:], e2[:], ALU.add)       # ee
            nc.vector.tensor_tensor(e1[:], e1[:], e2[:], ALU.add)         # err+ee
            nc.vector.tensor_tensor(s_[:], s_[:], e1[:], ALU.add)         # r2
            nc.vector.tensor_scalar(xyz[:, :, j], s_[:],
                                    kb[:, 3 * j + 2:3 * j + 3], DEPTH,
                                    ALU.add, ALU.mult)
        nc.sync.dma_start(pts_o.rearrange("(p c) k -> p (c k)", p=128),
                          xyz[:].rearrange("p c k -> p (c k)"))

        vi = ppool.tile([128, PPP], I32)
        cf = ppool.tile([128, PPP], F32)
        for src_j, gout in ((0, gxf), (1, gyf)):
            nc.vector.tensor_scalar(t0[:], xyz[:, :, src_j], 50.0, 2.0,
                                    ALU.add, ALU.mult)
            nc.vector.tensor_scalar(t0[:], t0[:], 0.0, 199.0, ALU.max, ALU.min)
            nc.vector.tensor_copy(vi[:], t0[:])
            nc.vector.tensor_copy(cf[:], vi[:])
            nc.vector.tensor_tensor(gout[:], cf[:], t0[:], ALU.is_gt)
            nc.vector.tensor_tensor(gout[:], cf[:], gout[:], ALU.subtract)
        nc.vector.scalar_tensor_tensor(flat[:], gyf[:], 200.0, gxf[:],
                                       ALU.mult, ALU.add)

        st = ppool.tile([128, 4], F32)
        stn = ppool.tile([128, 2], F32)
        nc.vector.tensor_reduce(stn[:, 0:1], gxf[:], axis=AXL.X, op=ALU.min)
        nc.vector.tensor_reduce(st[:, 1:2], gxf[:], axis=AXL.X, op=ALU.max)
        nc.vector.tensor_reduce(stn[:, 1:2], gyf[:], axis=AXL.X, op=ALU.min)
        nc.vector.tensor_reduce(st[:, 3:4], gyf[:], axis=AXL.X, op=ALU.max)
        nc.vector.tensor_scalar(st[:, 0:1], stn[:, 0:1], -1.0, None, ALU.mult)
        nc.vector.tensor_scalar(st[:, 2:3], stn[:, 1:2], -1.0, None, ALU.mult)
        stt_ps = psmall.tile([4, 128], F32, tag="aux")
        nc.tensor.transpose(stt_ps[:], st[:], ident[:])
        gst = ppool.tile([4, 1], F32)
        nc.vector.tensor_reduce(gst[:], stt_ps[:], axis=AXL.X, op=ALU.max)
        g_ps = psmall.tile([1, 4], F32, tag="aux")
        nc.tensor.matmul(g_ps[:], gst[:, 0:1], ident[0:4, 0:4],
                         start=True, stop=True)
        g_row = ppool.tile([1, 4], F32)
        nc.scalar.copy(g_row[:], g_ps[:])
        nc.sync.dma_start(flg_o, g_row[:])

        base1 = ppool.tile([1, 1], F32)
        nc.vector.scalar_tensor_tensor(base1[:], g_row[:, 2:3], 200.0,
                                       g_row[:, 0:1], ALU.mult, ALU.add)
        nc.vector.tensor_scalar(base1[:], base1[:], -1.0, None, ALU.mult)
        bb_ps = psmall.tile([128, 1], F32, tag="aux")
        nc.tensor.matmul(bb_ps[:], ones_r[:], base1[:], start=True, stop=True)
        baseb = ppool.tile([128, 1], F32)
        nc.scalar.copy(baseb[:], bb_ps[:])

        rel = ppool.tile([128, PPP], F32)
        nc.vector.tensor_scalar(rel[:], flat[:], baseb[:, 0:1], None, ALU.subtract)
        hits = ppool.tile([128, PROBE * PROBE], F32)
        scratch = ppool.tile([128, PPP], F32)
        for i in range(PROBE * PROBE):
            off = float((i // PROBE) * 200 + (i % PROBE))
            nc.vector.tensor_scalar(scratch[:], rel[:], off, 0.0,
                                    ALU.is_equal, ALU.add,
                                    accum_out=hits[:, i:i + 1])
        cnt_ps = psmall.tile([1, PROBE * PROBE], F32, tag="aux")
        nc.tensor.matmul(cnt_ps[:], ones_c[:], hits[:], start=True, stop=True)
        occ1 = ppool.tile([1, PROBE * PROBE], F32)
        nc.vector.tensor_scalar(occ1[:], cnt_ps[:], 0.0, None, ALU.is_gt)
        ob_ps = psmall.tile([128, PROBE * PROBE], F32, tag="aux")
        nc.tensor.matmul(ob_ps[:], ones_r[:], occ1[:], start=True, stop=True)
        occb = ppool.tile([128, PROBE * PROBE], F32)
        nc.scalar.copy(occb[:], ob_ps[:])

        cell_i = ppool.tile([128, BEV_COLS], I32)
        nc.gpsimd.iota(cell_i[:], pattern=[[1, BEV_COLS]], base=0,
                       channel_multiplier=BEV_COLS)
        cell_f = ppool.tile([128, BEV_COLS], F32)
        nc.vector.tensor_copy(cell_f[:], cell_i[:])
        relc = ppool.tile([128, BEV_COLS], F32)
        nc.vector.tensor_scalar(relc[:], cell_f[:], baseb[:, 0:1], None,
                                ALU.subtract)
        bev = ppool.tile([128, BEV_COLS], F32)
        nc.vector.memset(bev[:], 0.0)
        mk = ppool.tile([128, BEV_COLS], F32)
        for i in range(PROBE * PROBE):
            off = float((i // PROBE) * 200 + (i % PROBE))
            nc.vector.tensor_scalar(mk[:], relc[:], off, None, ALU.is_equal)
            nc.vector.scalar_tensor_tensor(bev[:], mk[:], occb[:, i:i + 1],
                                           bev[:], ALU.mult, ALU.add)
        nc.sync.dma_start(bev_o, bev[:])

        # ---------------- BN + conv2 + softmax ----------------
        ctx2.close()
        spool = ctx.enter_context(tc.tile_pool(name="smax", bufs=1))
        gstats = hpool.tile([128, 2], F32)
        nc.sync.dma_start(gstats[:], cc_out.ap())

        bn_sb = cpool.tile([128, 3], F32)
        nc.sync.dma_start(bn_sb[:], bnp)
        mu = hpool.tile([128, 1], F32)
        nc.vector.tensor_scalar(mu[:], gstats[:, 0:1], 1.0 / TOTAL_POS, None, ALU.mult)
        var = hpool.tile([128, 1], F32)
        nc.vector.tensor_scalar(var[:], gstats[:, 1:2], 1.0 / TOTAL_POS, None, ALU.mult)
        mu2 = hpool.tile([128, 1], F32)
        nc.vector.tensor_tensor(mu2[:], mu[:], mu[:], ALU.mult)
        nc.vector.tensor_tensor(var[:], var[:], mu2[:], ALU.subtract)
        sd = hpool.tile([128, 1], F32)
        eps_t = hpool.tile([128, 1], F32)
        nc.vector.memset(eps_t[:], float(EPS))
        nc.scalar.activation(sd[:], var[:], ACTF.Sqrt, bias=eps_t[:])
        rinv = hpool.tile([128, 1], F32)
        nc.vector.reciprocal(rinv[:], sd[:])
        scale = hpool.tile([128, 1], F32)
        nc.vector.tensor_tensor(scale[:], rinv[:], bn_sb[:, 1:2], ALU.mult)
        bias_f = hpool.tile([128, 1], F32)
        nc.vector.tensor_tensor(bias_f[:], mu[:], bn_sb[:, 0:1], ALU.add)
        nc.vector.tensor_tensor(bias_f[:], bias_f[:], scale[:], ALU.mult)
        nc.vector.tensor_tensor(bias_f[:], bn_sb[:, 2:3], bias_f[:], ALU.subtract)

        h_relu = hpool.tile([128, QTOT], F32R)
        nc.scalar.activation(h_relu[:], h_raw[:], ACTF.Relu,
                             bias=bias_f[:], scale=scale[:])

        w2_sb = cpool.tile([128, 64], F32R)
        nc.sync.dma_start(w2_sb[:], w2t)
        b2_sb = cpool.tile([64, 2], F32)
        nc.sync.dma_start(b2_sb[:], b2b)
        bins_r = cpool.tile([64, 1], F32R)
        nc.vector.tensor_copy(bins_r[:], b2_sb[:, 1:2])

        exp_t = spool.tile([64, QTOT], F32R)
        den = spool.tile([1, QTOT], F32R)
        num = spool.tile([1, QTOT], F32)
        rden = spool.tile([1, QTOT], F32)
        rden_r = spool.tile([1, QTOT], F32R)
        for c in range(NCHUNK):
            q0 = c * 512
            n = min(512, QTOT - q0)
            ps2 = psum.tile([64, 512], F32, tag="big")
            nc.tensor.matmul(ps2[:, 0:n], w2_sb[:], h_relu[:, q0:q0 + n],
                             start=True, stop=True)
            nc.scalar.activation(exp_t[:, q0:q0 + n], ps2[:, 0:n], ACTF.Exp,
                                 bias=b2_sb[:, 0:1], scale=1.0)
            psd = psmall.tile([1, 512], F32, tag="dn")
            nc.tensor.matmul(psd[:, 0:n], ones64r[:], exp_t[:, q0:q0 + n],
                             start=True, stop=True)
            nc.scalar.copy(den[:, q0:q0 + n], psd[:, 0:n])
            psn = psmall.tile([1, 512], F32, tag="dn")
            nc.tensor.matmul(psn[:, 0:n], bins_r[:], exp_t[:, q0:q0 + n],
                             start=True, stop=True)
            nc.scalar.copy(num[:, q0:q0 + n], psn[:, 0:n])
            nc.vector.reciprocal(rden[:, q0:q0 + n], den[:, q0:q0 + n].bitcast(F32))
            nc.scalar.copy(rden_r[:, q0:q0 + n], rden[:, q0:q0 + n])

        ed1 = num
        nc.vector.tensor_tensor(ed1[:], num[:], rden[:], ALU.mult)
        nc.sync.dma_start(
            ed_o.rearrange("one (h w) -> one h w", h=HW_ROWS),
            ed1[:].rearrange("one (h w) -> one h w", h=HW_ROWS)[:, :, 0:FW])

        probs = spool.tile([64, QTOT], F32)
        for c in range(NCHUNK):
            q0 = c * 512
            n = min(512, QTOT - q0)
            psr = psum.tile([64, 512], F32, tag="big")
            nc.tensor.matmul(psr[:, 0:n], onesrow_r[:], rden_r[:, q0:q0 + n],
                             start=True, stop=True)
            nc.vector.tensor_tensor(probs[:, q0:q0 + n],
                                    exp_t[:, q0:q0 + n].bitcast(F32),
                                    psr[:, 0:n], ALU.mult)
        nc.sync.dma_start(
            dd_o.rearrange("d (h w) -> d h w", h=HW_ROWS),
            probs[:].rearrange("d (h w) -> d h w", h=HW_ROWS)[:, :, 0:FW])

    nc.compile()
    return nc


_NC_CACHE = None


def kernel(camera_features, pixels_uv, K_inv, W1, b1, gamma, beta, W2, b2,
           depth_bins):
    global _NC_CACHE
    if _NC_CACHE is None:
        _NC_CACHE = _build()
    nc = _NC_CACHE

    camera_features = np.asarray(camera_features, dtype=np.float32)
    pixels_uv = np.ascontiguousarray(np.asarray(pixels_uv, dtype=np.float32))
    K_inv = np.asarray(K_inv, dtype=np.float32)
    W1 = np.asarray(W1, dtype=np.float32)
    b1 = np.asarray(b1, dtype=np.float32)
    gamma = np.asarray(gamma, dtype=np.float32)
    beta = np.asarray(beta, dtype=np.float32)
    W2 = np.asarray(W2, dtype=np.float32)
    b2 = np.asarray(b2, dtype=np.float32)
    depth_bins = np.asarray(depth_bins, dtype=np.float32)

    # host-side layout prep (pure data movement)
    w1t = np.empty((18, 128, 128), np.float32)
    for ky in range(3):
        for kx in range(3):
            for kh in range(2):
                # [ci, co] for tap (ky,kx), K-half kh
                w1t[(ky * 3 + kx) * 2 + kh] = \
                    W1[:, kh * 128:(kh + 1) * 128, ky, kx].T
    xp = np.zeros((B, C_IN, FH + 2, WPAD), np.float32)
    xp[:, :, 1:FH + 1, 1:FW + 1] = camera_features
    w2t = np.ascontiguousarray(W2[:, :, 0, 0].T)
    bnp = np.ascontiguousarray(np.stack([b1, gamma, beta], axis=1))
    b2bins = np.ascontiguousarray(np.stack([b2, depth_bins], axis=1))

    in_maps = []
    for c in range(N_CORES):
        b = c // 2
        half = c % 2
        r0 = half * HW_ROWS
        uv_sl = pixels_uv[b, half * PPC:(half + 1) * PPC]
        uv_sh = np.concatenate([uv_sl, uv_sl[:PPC_PAD - PPC]], axis=0)
        in_maps.append({
            "xpad": np.ascontiguousarray(
                xp[b, :, r0:r0 + 34, :]).reshape(C_IN, 34 * WPAD),
            "uv": np.ascontiguousarray(uv_sh),
            "kinv": K_inv[b].reshape(1, 9).copy(),
            "w1t": w1t,
            "w2t": w2t,
            "bnp": bnp,
            "b2b": b2bins,
        })

    trace = bool(getattr(kernel, "_trace", False))
    res = run_bass_kernel_spmd(nc, in_maps, core_ids=list(range(N_CORES)),
                               trace=trace)
    kernel._last_exec_ns = res.exec_time_ns
    kernel._last_results = res

    dd = np.empty((B, D, FH, FW), np.float32)
    ed = np.empty((B, FH, FW), np.float32)
    pts = np.empty((B, N_PTS, 3), np.float32)
    bev = np.empty((B, 200, 200), np.float32)
    fallback = False
    for c in range(N_CORES):
        b = c // 2
        half = c % 2
        r0 = half * HW_ROWS
        r = res.results[c]
        dd[b, :, r0:r0 + HW_ROWS, :] = r["dd"].reshape(D, HW_ROWS, FW)
        ed[b, r0:r0 + HW_ROWS, :] = r["ed"].reshape(HW_ROWS, FW)
        pts[b, half * PPC:(half + 1) * PPC] = r["pts"][:PPC]
        g = r["bev"].ravel()[:40000].reshape(200, 200)
        flg = r["flg"].ravel()  # [-minx, maxx, -miny, maxy]
        if (flg[1] + flg[0] > PROBE - 1) or (flg[3] + flg[2] > PROBE - 1):
            fallback = True
        if half == 0:
            bev[b] = g
        else:
            np.maximum(bev[b], g, out=bev[b])

    if fallback:
        # exact host fallback (never taken for the target input distribution)
        for b in range(B):
            gx = np.clip(((pts[b, :, 0] + 50.0) / 0.5).astype(np.int32), 0, 199)
            gy = np.clip(((pts[b, :, 1] + 50.0) / 0.5).astype(np.int32), 0, 199)
            grid = np.zeros(40000, np.float32)
            np.add.at(grid, gy * 200 + gx, 1.0)
            bev[b] = np.clip(grid, 0.0, 1.0).reshape(200, 200)

    return bev, dd, ed, pts
